# revision 29
# baseline (speedup 1.0000x reference)
"""Trainium2 Bass kernel for Llama SmartKV decode attention (GQA, q_len=1).

Sharding: tensor-parallel over KV heads — core c owns kv head c and its
GQA group of 4 query heads (slices of Wq/Wk/Wv/Wo), plus that head's
quantized KV cache. Each core computes its partial o_proj output; the
host sums the 8 partials (the all-reduce).

Byte-budget design (per core, the binding resources):
  - Projection weights are int8 in DRAM with per-ROW scales folded into
    the host-prepped hsT (wqkv) and the PV-descale vector (wo), so no
    on-chip scale corrections are needed.  wqkv is DMA'd raw (HWDGE) and
    upcast to fp16 on DVE/ACT (engine ports, not the DMA fabric); wo is
    SWDGE-cast late when the fabric is idle.
  - KV cache codes are stored as fp8e3 (E3M4) = codes/16 (exact range
    +-7.94 within E3M4's +-15.5), read raw over HWDGE with no cast, and
    fed to the PE as the fp8 stationary operand (halves LDWEIGHTS time).
    The x16 is folded into k_scale/v_scale (stored fp16).
  - One HWDGE FIFO orders the big streams (hsT, wqkv, kT, v8) with no
    inter-stream dep sems; wo streams last so o_proj pipelines per-chunk
    behind it, shrinking the tail to ~2-3us.
HBM ~13.9MB (the 358GB/s floor), SBUF-fabric ~15.9MB, PE ~36us.
"""

import os

os.environ.setdefault("BY_DEFAULT_DISABLE_SUBTILE_DEPS", "1")

import ml_dtypes
import numpy as np

import concourse.bass as bass
import concourse.mybir as mybir
import concourse.tile as tile
from concourse.bass_utils import run_bass_kernel_spmd
from concourse.tile_rust import add_dep_helper

H, HKV, D, HID, S = 32, 8, 128, 4096, 32768
G = H // HKV  # 4 query heads per core
NCORES = 8
KC = HID // 128  # 32 contraction chunks for projections
KTILE = 4096  # tokens per K-cache DMA tile
VCH = 32  # s-chunks per V-cache DMA tile
F16 = mybir.dt.float16
F8E3 = mybir.dt.float8e3
I8 = mybir.dt.int8
F32 = mybir.dt.float32
EXP_BIAS = -9.0  # exp(s + B): cancels in softmax, keeps fp16 in range
CHS = 2.0**6  # hs pre-scale: keeps hsT entries in fp16 normal range
F8S = 16.0  # fp8e3 cache codes are stored as codes/16
RSQD = 1.0 / np.sqrt(np.float32(D))  # score scale (NOT folded into Wq:
# folding it would shrink Wq 11x vs Wk/Wv and waste the shared per-row
# int8 levels; applied via ksc and the scur activation scale instead)

_CACHE = {}


def _reduce_dma_waits(nc):
    """Drop transitively-implied waits from instructions.

    The PSEUDO_DMA_DIRECT2D descriptor holds exactly one wait slot, but
    Tile's sem assignment is not transitively minimal (its optimize_sems
    pass is disabled), so pool-slot-recycling DMAs carry a redundant
    second wait: the WAW wait on the previous slot writer is already
    implied by the engine-reader wait.  We verify implication with a
    vector-clock walk over the scheduled program and delete only waits
    that are provably redundant.
    """
    import bass_rust as _br

    insts = []
    for f in nc.m.functions:
        for bb in f.blocks:
            insts.extend(bb.instructions)

    cum = {}  # sem name -> cumulative value so far in schedule order
    snaps = {}  # sem name -> list of (cumval, knowledge dict)
    streams = {}  # stream key -> knowledge dict (sem name -> value known >=)

    def know_at(sem, val):
        # knowledge of the producer that first brought `sem` to >= val
        for cv, kn in snaps.get(sem, ()):
            if cv >= val:
                return kn
        return None

    for inst in insts:
        si = inst.sync_info
        if si is None:
            continue
        waits = list(si.on_wait)
        ups = list(si.on_update)
        if ups and ups[0].ant_name.startswith(("DMASW", "DMAHW")):
            skey = ups[0].ant_name
        else:
            skey = f"eng:{inst.engine}"
        kn = dict(streams.get(skey, ()))

        imm = [
            w
            for w in waits
            if w.wait_mode == "sem-ge-imm" and w.sync_type == "semaphore"
        ]
        if len(imm) == len(waits) > 1:
            keep = []
            for w in waits:
                others = dict(kn)
                for w2 in waits:
                    if w2 is w:
                        continue
                    others[w2.ant_name] = max(
                        others.get(w2.ant_name, 0), w2.wait_value
                    )
                    k2 = know_at(w2.ant_name, w2.wait_value)
                    if k2:
                        for s, v in k2.items():
                            others[s] = max(others.get(s, 0), v)
                if others.get(w.ant_name, 0) >= w.wait_value:
                    continue  # implied: drop
                keep.append(w)
            if len(keep) < len(waits):
                inst.sync_info = _br.SyncInfo(on_wait=keep, on_update=ups)
                waits = keep

        # fold wait knowledge into this instruction's stream knowledge
        for w in waits:
            if w.wait_mode != "sem-ge-imm" or w.sync_type != "semaphore":
                continue
            kn[w.ant_name] = max(kn.get(w.ant_name, 0), w.wait_value)
            k2 = know_at(w.ant_name, w.wait_value)
            if k2:
                for s, v in k2.items():
                    kn[s] = max(kn.get(s, 0), v)
        for u in ups:
            if u.sync_type != "semaphore":
                continue
            cum[u.ant_name] = cum.get(u.ant_name, 0) + u.update_value
            kn[u.ant_name] = max(kn.get(u.ant_name, 0), cum[u.ant_name])
            snaps.setdefault(u.ant_name, []).append((cum[u.ant_name], kn))
        streams[skey] = kn

    bad = [
        (i.name, type(i).__name__, [(w.ant_name, w.wait_value) for w in i.sync_info.on_wait])
        for i in insts
        if i.sync_info is not None
        and len(i.sync_info.on_wait) > 1
        and type(i).__name__ not in ("InstDrain",)
    ]
    if bad:
        print(f"WARNING: {len(bad)} instructions still multi-wait: {bad[:6]}")


def _build_nc():
    nc = bass.Bass()
    hsT = nc.declare_dram_parameter("hsT", [128, KC], F16, isOutput=False)
    swo2 = nc.declare_dram_parameter("swo2", [128, G], F32, isOutput=False)
    wqkv = nc.declare_dram_parameter("wqkv", [8, 128, 4 * 768], I8, isOutput=False)
    kT = nc.declare_dram_parameter("kT", [128, S], I8, isOutput=False)
    ksc = nc.declare_dram_parameter("ksc", [128, S // 128], F32, isOutput=False)
    vsc = nc.declare_dram_parameter("vsc", [128, S // 128], F16, isOutput=False)
    v8 = nc.declare_dram_parameter("v8", [S // (128 * VCH), 128, VCH * D], F8E3, isOutput=False)
    wo8 = nc.declare_dram_parameter("wo8", [8, 128, G * 512], I8, isOutput=False)
    out = nc.declare_dram_parameter("out", [1, HID], F32, isOutput=True)

    PS = bass.MemorySpace.PSUM
    with tile.TileContext(nc) as tc:
        with (
            tc.tile_pool(name="const", bufs=1) as cpool,
            tc.tile_pool(name="w8p", bufs=4) as w8_pool,
            tc.tile_pool(name="w16ap", bufs=4) as w16a_pool,
            tc.tile_pool(name="w16bp", bufs=4) as w16b_pool,
            tc.tile_pool(name="k16p", bufs=8) as k16_pool,
            tc.tile_pool(name="vp", bufs=8) as v_pool,
            tc.tile_pool(name="wo8p", bufs=4) as wo8_pool,
            tc.tile_pool(name="wop", bufs=4) as wo_pool,
            tc.tile_pool(name="sm", bufs=1) as sm,
        ):
            # ---- constants + tiny loads (sync FIFO head) ----
            hs_sb = cpool.tile([128, KC], F16)
            nc.sync.dma_start(out=hs_sb, in_=hsT[:, :])
            swo_sb = cpool.tile([128, G], F32)
            nc.sync.dma_start(out=swo_sb, in_=swo2[:, :])
            ebias_sb = cpool.tile([128, 1], F32)
            nc.vector.memset(ebias_sb, EXP_BIAS)
            ones_sb = cpool.tile([128, 1], F32)
            nc.vector.memset(ones_sb, 1.0)
            ones_row = cpool.tile([1, 128], F32)
            nc.vector.memset(ones_row, 1.0)
            warm_sb = cpool.tile([128, 512], F16)
            nc.vector.memset(warm_sb, 0.0)
            zero16_sb = cpool.tile([128, 1], F16)
            nc.vector.memset(zero16_sb, 0.0)

            qk_sb = sm.tile([128, G + 1], F16)
            vrow_sb = sm.tile([1, D], F32)
            ksc_sb = cpool.tile([128, S // 128], F32)
            nc.sync.dma_start(out=ksc_sb, in_=ksc[:, :])
            vsc_sb = cpool.tile([128, S // 128], F16)
            nc.sync.dma_start(out=vsc_sb, in_=vsc[:, :])
            # tiny DVE reads so later DVE ops inherit the scale-DMA waits
            # through the engine stream (keeps every op single-wait)
            touch_sb = sm.tile([1, 2], F32)
            nc.vector.tensor_copy(out=touch_sb[:, 0:1], in_=ksc_sb[0:1, 0:1])
            nc.vector.tensor_copy(out=touch_sb[:, 1:2], in_=vsc_sb[0:1, 0:1])

            # ---- q/k/v projections; wqkv arrives int8, upcast on DVE/ACT ----
            with tc.tile_pool(name="psqk", bufs=1, space=PS) as psqk_pool:
                # HAM warm-up: keep PE busy through its cold window while
                # the first weight tiles are still in flight
                ps_warm = psqk_pool.tile([128, 512], F32, tag="warm")
                for _ in range(10):
                    nc.tensor.matmul(
                        ps_warm[:, :], lhsT=warm_sb[:, 0:128], rhs=warm_sb,
                        start=True, stop=True,
                    )

                ps_qk = [
                    psqk_pool.tile([128, 1], F32, name=f"ps_qk{h}", tag=f"qk{h}")
                    for h in range(G + 1)
                ]
                ps_v = psqk_pool.tile([1, D], F32, tag="psv")
                # wqkv tile layout: 24 col-blocks of 128 = (c, h) with
                # block = 6c + h.  DVE upcasts blocks 0-15, ACT 16-23
                # (separate dest tiles so every proj MM waits one writer).
                proj_last = []  # last proj MM per tile (threads pool WARs)
                for t in range(8):
                    w8_sb = w8_pool.tile([128, 4 * 768], I8, tag="w8")
                    wd = nc.sync.dma_start(out=w8_sb, in_=wqkv[t])
                    if t >= 4:
                        add_dep_helper(wd.ins, proj_last[t - 4].ins, sync=True,
                                       reason="w8/w16 slots free before reuse")
                    w16a_sb = w16a_pool.tile([128, 2048], F16, tag="w16a")
                    w16b_sb = w16b_pool.tile([128, 1024], F16, tag="w16b")
                    # tensor_tensor (not copy/cast) stays off the shared DVE
                    # port so GPSIMD SWDGE descriptor-gen is never starved
                    z_bc = bass.AP(tensor=zero16_sb.tensor, offset=zero16_sb.offset,
                                   ap=[zero16_sb.ap[0], [0, 2048]])
                    nc.vector.tensor_add(out=w16a_sb, in0=w8_sb[:, 0:2048], in1=z_bc)
                    nc.scalar.copy(out=w16b_sb, in_=w8_sb[:, 2048:3072])

                    def wslice(c, h):
                        blk = 6 * c + h
                        if blk < 16:
                            return w16a_sb[:, blk * 128 : (blk + 1) * 128]
                        return w16b_sb[:, (blk - 16) * 128 : (blk - 15) * 128]

                    for c in range(4):
                        j = 4 * t + c
                        for h in range(G + 1):
                            nc.tensor.matmul(
                                ps_qk[h][:, :],
                                lhsT=wslice(c, h),
                                rhs=hs_sb[:, j : j + 1],
                                start=(j == 0),
                                stop=(j == KC - 1),
                            )
                        vmm = nc.tensor.matmul(
                            ps_v[:, :],
                            lhsT=hs_sb[:, j : j + 1],
                            rhs=wslice(c, 5),
                            start=(j == 0),
                            stop=(j == KC - 1),
                        )
                    proj_last.append(vmm)
                for h in range(G + 1):
                    nc.vector.tensor_copy(out=qk_sb[:, h : h + 1], in_=ps_qk[h])
                # 2^14 matches the vsc-folded PV accumulation; /CHS undoes
                # the hsT pre-scale
                nc.scalar.mul(out=vrow_sb, in_=ps_v, mul=16384.0 / CHS)

            with (
                tc.tile_pool(name="pssc", bufs=1, space=PS) as pssc_pool,
                tc.tile_pool(name="pspv", bufs=1, space=PS) as pspv_pool,
                tc.tile_pool(name="psms", bufs=1, space=PS) as psms_pool,
            ):
                # ---- current-token score row: s_curT[1, g] = k_cur . q_g ----
                # q and k both carry a CHS factor from hsT -> scale 1/CHS^2
                ps_scur = psms_pool.tile([1, G], F32, tag="ms")
                nc.tensor.matmul(
                    ps_scur[:, :], lhsT=qk_sb[:, G : G + 1], rhs=qk_sb[:, 0:G],
                    start=True, stop=True,
                )
                pcurf_sb = sm.tile([1, G], F32)
                nc.scalar.activation(
                    out=pcurf_sb, in_=ps_scur,
                    func=mybir.ActivationFunctionType.Exp, bias=ebias_sb[:1],
                    scale=float(RSQD / (CHS * CHS)),
                )

                # ---- scores over the cache: [s, g] layout, 2 PSUM banks ----
                ps_sc = [
                    pssc_pool.tile([128, 512], F32, name=f"ps_sc{b}", tag=f"sc{b}")
                    for b in range(2)
                ]
                probs_sb = [
                    sm.tile([128, 512], F16, name=f"probs{b}", tag=f"pr{b}")
                    for b in range(2)
                ]
                pprime_sb = [
                    sm.tile([128, 512], F16, name=f"pprime{b}", tag=f"pp{b}")
                    for b in range(2)
                ]
                dpart_sb = sm.tile([128, 2 * G], F32)
                kcpt = KTILE // 128  # score chunks per K tile
                for co in range(S // KTILE):
                    # SWDGE cast int8->fp16 straight into the PE-ready tile:
                    # no engine upcast, and the fp16 fabric cost rides the
                    # otherwise idle early SBUF-AXI budget
                    k16_sb = k16_pool.tile([128, KTILE], F16, tag="k16")
                    nc.gpsimd.dma_start(
                        out=k16_sb, in_=kT[:, co * KTILE : (co + 1) * KTILE]
                    )
                    for ci in range(kcpt):
                        ch = co * kcpt + ci
                        b, col = ch // 128, (ch % 128) * 4
                        nc.tensor.matmul(
                            ps_sc[b][:, col : col + 4],
                            lhsT=k16_sb[:, ci * 128 : (ci + 1) * 128],
                            rhs=qk_sb[:, 0:G],
                            start=True,
                            stop=True,
                        )
                    if (co + 1) * kcpt % 128 == 0:
                        b = ((co + 1) * kcpt - 1) // 128
                        # scores = raw_codes_dot * k_scale[s]  (per-s scale,
                        # broadcast over the 4 g columns)
                        kb = ksc_sb[:, b * 128 : (b + 1) * 128]
                        kb_bc = bass.AP(tensor=kb.tensor, offset=kb.offset,
                                        ap=[*kb.ap, [0, G]])
                        scraw = sm.tile([128, 512], F32, name=f"scraw{b}", tag="scr", bufs=2)
                        nc.vector.tensor_mul(
                            out=scraw.rearrange("p (c g) -> p c g", g=G),
                            in0=ps_sc[b].rearrange("p (c g) -> p c g", g=G),
                            in1=kb_bc,
                        )
                        nc.scalar.activation(
                            out=probs_sb[b], in_=scraw,
                            func=mybir.ActivationFunctionType.Exp, bias=ebias_sb,
                        )
                        # per-(partition, g) partials: reduce over the 128
                        # chunk-columns (stride 4) of the bank
                        nc.vector.reduce_sum(
                            out=dpart_sb[:, b * G : (b + 1) * G],
                            in_=probs_sb[b].rearrange("p (c g) -> p g c", g=G),
                            axis=mybir.AxisListType.X,
                        )
                        # fold v_scale[s] into the probabilities used by PV
                        vb = vsc_sb[:, b * 128 : (b + 1) * 128]
                        vb_bc = bass.AP(tensor=vb.tensor, offset=vb.offset,
                                        ap=[*vb.ap, [0, G]])
                        nc.vector.tensor_mul(
                            out=pprime_sb[b].rearrange("p (c g) -> p c g", g=G),
                            in0=probs_sb[b].rearrange("p (c g) -> p c g", g=G),
                            in1=vb_bc,
                        )

                # ---- denominator; rden broadcast; fold wo row scales ----
                ps_den = psms_pool.tile([1, 2 * G], F32, tag="ms")
                nc.tensor.matmul(
                    ps_den[:, :], lhsT=ones_sb, rhs=dpart_sb, start=True, stop=True,
                )
                den_sb = sm.tile([1, 2 * G], F32)
                nc.vector.tensor_copy(out=den_sb, in_=ps_den)
                dtot_sb = sm.tile([1, G], F32)
                nc.vector.tensor_add(
                    out=dtot_sb, in0=den_sb[:, 0:G], in1=den_sb[:, G : 2 * G]
                )
                nc.vector.tensor_add(out=dtot_sb, in0=dtot_sb, in1=pcurf_sb)
                rden_sb = sm.tile([1, G], F32)
                nc.vector.reciprocal(out=rden_sb, in_=dtot_sb)
                # broadcast rden across partitions on PE (ones outer product)
                ps_bc = psms_pool.tile([128, G], F32, tag="ms")
                nc.tensor.matmul(
                    ps_bc[:, :], lhsT=ones_row, rhs=rden_sb, start=True, stop=True
                )
                bc_sb = sm.tile([128, G], F32)
                # bc = rden[g] * wo_row_scale[p,g] * 2^-14
                nc.vector.tensor_mul(out=bc_sb, in0=ps_bc, in1=swo_sb)

                # ---- PV: outT[d, g] over all 256 chunks + current token ----
                v_dmas = []
                ps_pv = pspv_pool.tile([128, G], F32)
                for co in range(S // (128 * VCH)):
                    v_sb = v_pool.tile([128, VCH * D], F8E3, tag="vt")
                    vd = nc.sync.dma_start(out=v_sb, in_=v8[co])
                    v_dmas.append(vd)
                    for ci in range(VCH):
                        ch = co * VCH + ci
                        b, col = ch // 128, (ch % 128) * 4
                        nc.tensor.matmul(
                            ps_pv[:, :],
                            lhsT=v_sb[:, ci * D : (ci + 1) * D],
                            rhs=pprime_sb[b][:, col : col + 4],
                            start=(ch == 0),
                            stop=False,
                        )
                nc.tensor.matmul(
                    ps_pv[:, :], lhsT=vrow_sb, rhs=pcurf_sb, start=False, stop=True,
                )
                outn_sb = sm.tile([128, G], F16)
                nc.vector.tensor_mul(out=outn_sb, in0=ps_pv, in1=bc_sb)

            # ---- o_proj, pipelined per 512-col chunk behind the wo stream ----
            with tc.tile_pool(name="pso", bufs=2, space=PS) as pso_pool:
                ofin_sb = sm.tile([1, HID], F32)
                ofin_copies = []
                for n in range(8):
                    wo8_sb = wo8_pool.tile([128, G * 512], I8, tag="wo8")
                    wd = nc.sync.dma_start(out=wo8_sb, in_=wo8[n])
                    if n >= 2:
                        # thread the ps_on / wo16-slot recycle WARs through
                        # the wo DMA so everything downstream is single-wait
                        add_dep_helper(wd.ins, ofin_copies[n - 2].ins, sync=True,
                                       reason="ps_on bank free before wo lands")
                    wo_sb = wo_pool.tile([128, G * 512], F16, tag="wo")
                    if n in (1, 5):
                        nc.scalar.copy(out=wo_sb, in_=wo8_sb)
                    else:
                        nc.vector.tensor_copy(out=wo_sb, in_=wo8_sb)
                    ps_on = pso_pool.tile([1, 512], F32, tag="on")
                    for g in range(G):
                        nc.tensor.matmul(
                            ps_on[:, :],
                            lhsT=outn_sb[:, g : g + 1],
                            rhs=wo_sb[:, g * 512 : (g + 1) * 512],
                            start=(g == 0),
                            stop=(g == G - 1),
                        )
                    ofin_copies.append(
                        nc.scalar.copy(out=ofin_sb[:, n * 512 : (n + 1) * 512], in_=ps_on)
                    )
            nc.gpsimd.dma_start(out=out[:, :], in_=ofin_sb)

    _reduce_dma_waits(nc)
    return nc


def _rope_fold(W, nheads, cos, sin, scale=1.0):
    """Fold RoPE rotation (and an optional scalar) into projection weights."""
    W = W.reshape(HID, nheads, D).astype(np.float32)
    half = D // 2
    Wr = np.empty_like(W)
    Wr[:, :, :half] = cos[:half] * W[:, :, :half] - sin[:half] * W[:, :, half:]
    Wr[:, :, half:] = cos[half:] * W[:, :, half:] + sin[half:] * W[:, :, :half]
    return (Wr * np.float32(scale)).reshape(HID, nheads * D)


def _prep_inputs(hidden_states, k_qx, k_scale, v_qx, v_scale, cos, sin, Wq, Wk, Wv, Wo):
    f16 = np.float16
    f8 = ml_dtypes.float8_e3m4
    hs = np.ascontiguousarray(hidden_states.reshape(HID)).astype(np.float32)
    cos = cos.astype(np.float32)
    sin = sin.astype(np.float32)
    Wq_f = _rope_fold(Wq, H, cos, sin)  # 1/sqrt(D) lives in ksc / scur scale
    Wk_f = _rope_fold(Wk, HKV, cos, sin)

    in_maps = []
    for c in range(NCORES):
        qcols = slice(G * c * D, G * (c + 1) * D)
        kvcols = slice(c * D, (c + 1) * D)
        Wsl = np.concatenate(
            [Wq_f[:, qcols], Wk_f[:, kvcols], Wv[:, kvcols].astype(np.float32)],
            axis=1,
        )  # [HID, 768]
        srow = np.maximum(np.abs(Wsl).max(axis=1) / 127.0, 1e-12)
        w8 = np.clip(np.rint(Wsl / srow[:, None]), -127, 127).astype(np.int8)
        wqkv = np.ascontiguousarray(
            w8.reshape(8, 4, 128, 768).transpose(0, 2, 1, 3)
        ).reshape(8, 128, 4 * 768)
        hsT = np.ascontiguousarray(
            (hs * srow * CHS).reshape(KC, 128).T
        ).astype(f16)

        kT = np.ascontiguousarray(k_qx[:, c, :].astype(np.int8).T)
        vcodes = v_qx[:, c, :].astype(np.float32)
        vf8 = (vcodes / F8S).astype(f8)
        v8a = np.ascontiguousarray(
            vf8.reshape(S // (128 * VCH), VCH, 128, D).transpose(0, 2, 1, 3)
        ).reshape(S // (128 * VCH), 128, VCH * D)
        # per-token LSQ refit of v_scale against the fp8-rounded codes
        vhat = vf8.astype(np.float32) * F8S
        adj = (vcodes * vhat).sum(1) / np.maximum((vhat * vhat).sum(1), 1e-9)
        ksc = np.ascontiguousarray(
            (k_scale[:, c, 0].astype(np.float32) * RSQD / CHS)
            .reshape(S // 128, 128).T
        ).astype(np.float32)
        vsc = np.ascontiguousarray(
            (v_scale[:, c, 0].astype(np.float32) * adj * F8S * 16384.0)
            .reshape(S // 128, 128).T
        ).astype(f16)

        Wol = Wo[G * c * D : G * (c + 1) * D, :].astype(np.float32)  # [512, HID]
        srow_o = np.maximum(np.abs(Wol).max(axis=1) / 127.0, 1e-12)
        wo8f = np.clip(np.rint(Wol / srow_o[:, None]), -127, 127).astype(np.int8)
        wo8 = np.ascontiguousarray(
            wo8f.reshape(G, 128, 8, 512).transpose(2, 1, 0, 3)
        ).reshape(8, 128, G * 512)
        swo2 = np.ascontiguousarray(
            (srow_o / 16384.0).reshape(G, 128).T
        ).astype(np.float32)

        in_maps.append(
            {"hsT": hsT, "swo2": swo2, "wqkv": wqkv, "kT": kT, "v8": v8a,
             "wo8": wo8, "ksc": ksc, "vsc": vsc}
        )
    return in_maps


def _run(in_maps, trace=False, **kw):
    if "nc" not in _CACHE:
        _CACHE["nc"] = _build_nc()
    return run_bass_kernel_spmd(
        _CACHE["nc"], in_maps, core_ids=list(range(NCORES)), trace=trace, **kw
    )


def kernel(hidden_states, k_qx, k_scale, v_qx, v_scale, cos, sin, Wq, Wk, Wv, Wo):
    in_maps = _prep_inputs(
        hidden_states, k_qx, k_scale, v_qx, v_scale, cos, sin, Wq, Wk, Wv, Wo
    )
    res = _run(in_maps)
    out = np.zeros((1, 1, HID), np.float32)
    for r in res.results:
        out += r["out"].reshape(1, 1, HID)
    return out


# revision 31
# speedup vs baseline: 1.0601x; 1.0601x over previous
"""Trainium2 Bass kernel for Llama SmartKV decode attention (GQA, q_len=1).

Sharding: tensor-parallel over KV heads — core c owns kv head c and its
GQA group of 4 query heads (slices of Wq/Wk/Wv/Wo), plus that head's
quantized KV cache. Each core computes its partial o_proj output; the
host sums the 8 partials (the all-reduce).

Byte-budget design (per core, the binding resources):
  - Projection weights are int8 in DRAM with per-ROW scales folded into
    the host-prepped hsT (wqkv) and the PV-descale vector (wo), so no
    on-chip scale corrections are needed.  wqkv is DMA'd raw (HWDGE) and
    upcast to fp16 on DVE/ACT (engine ports, not the DMA fabric); wo is
    SWDGE-cast late when the fabric is idle.
  - KV cache codes are stored as fp8e3 (E3M4) = codes/16 (exact range
    +-7.94 within E3M4's +-15.5), read raw over HWDGE with no cast, and
    fed to the PE as the fp8 stationary operand (halves LDWEIGHTS time).
    The x16 is folded into k_scale/v_scale (stored fp16).
  - One HWDGE FIFO orders the big streams (hsT, wqkv, kT, v8) with no
    inter-stream dep sems; wo streams last so o_proj pipelines per-chunk
    behind it, shrinking the tail to ~2-3us.
HBM ~13.9MB (the 358GB/s floor), SBUF-fabric ~15.9MB, PE ~36us.
"""

import os

os.environ.setdefault("BY_DEFAULT_DISABLE_SUBTILE_DEPS", "1")

import ml_dtypes
import numpy as np

import concourse.bass as bass
import concourse.mybir as mybir
import concourse.tile as tile
from concourse.bass_utils import run_bass_kernel_spmd
from concourse.tile_rust import add_dep_helper

H, HKV, D, HID, S = 32, 8, 128, 4096, 32768
G = H // HKV  # 4 query heads per core
NCORES = 8
KC = HID // 128  # 32 contraction chunks for projections
KTILE = 4096  # tokens per K-cache DMA tile
VCH = 32  # s-chunks per V-cache DMA tile
F16 = mybir.dt.float16
F8E3 = mybir.dt.float8e3
I8 = mybir.dt.int8
F32 = mybir.dt.float32
EXP_BIAS = -9.0  # exp(s + B): cancels in softmax, keeps fp16 in range
CHS = 2.0**6  # hs pre-scale: keeps hsT entries in fp16 normal range
F8S = 16.0  # fp8e3 cache codes are stored as codes/16
RSQD = 1.0 / np.sqrt(np.float32(D))  # score scale (NOT folded into Wq:
# folding it would shrink Wq 11x vs Wk/Wv and waste the shared per-row
# int8 levels; applied via ksc and the scur activation scale instead)

_CACHE = {}


def _reduce_dma_waits(nc):
    """Drop transitively-implied waits from instructions.

    The PSEUDO_DMA_DIRECT2D descriptor holds exactly one wait slot, but
    Tile's sem assignment is not transitively minimal (its optimize_sems
    pass is disabled), so pool-slot-recycling DMAs carry a redundant
    second wait: the WAW wait on the previous slot writer is already
    implied by the engine-reader wait.  We verify implication with a
    vector-clock walk over the scheduled program and delete only waits
    that are provably redundant.
    """
    import bass_rust as _br

    insts = []
    for f in nc.m.functions:
        for bb in f.blocks:
            insts.extend(bb.instructions)

    cum = {}  # sem name -> cumulative value so far in schedule order
    snaps = {}  # sem name -> list of (cumval, knowledge dict)
    streams = {}  # stream key -> knowledge dict (sem name -> value known >=)

    def know_at(sem, val):
        # knowledge of the producer that first brought `sem` to >= val
        for cv, kn in snaps.get(sem, ()):
            if cv >= val:
                return kn
        return None

    for inst in insts:
        si = inst.sync_info
        if si is None:
            continue
        waits = list(si.on_wait)
        ups = list(si.on_update)
        if ups and ups[0].ant_name.startswith(("DMASW", "DMAHW")):
            skey = ups[0].ant_name
        else:
            skey = f"eng:{inst.engine}"
        kn = dict(streams.get(skey, ()))

        imm = [
            w
            for w in waits
            if w.wait_mode == "sem-ge-imm" and w.sync_type == "semaphore"
        ]
        if len(imm) == len(waits) > 1:
            keep = []
            for w in waits:
                others = dict(kn)
                for w2 in waits:
                    if w2 is w:
                        continue
                    others[w2.ant_name] = max(
                        others.get(w2.ant_name, 0), w2.wait_value
                    )
                    k2 = know_at(w2.ant_name, w2.wait_value)
                    if k2:
                        for s, v in k2.items():
                            others[s] = max(others.get(s, 0), v)
                if others.get(w.ant_name, 0) >= w.wait_value:
                    continue  # implied: drop
                keep.append(w)
            if len(keep) < len(waits):
                inst.sync_info = _br.SyncInfo(on_wait=keep, on_update=ups)
                waits = keep

        # fold wait knowledge into this instruction's stream knowledge
        for w in waits:
            if w.wait_mode != "sem-ge-imm" or w.sync_type != "semaphore":
                continue
            kn[w.ant_name] = max(kn.get(w.ant_name, 0), w.wait_value)
            k2 = know_at(w.ant_name, w.wait_value)
            if k2:
                for s, v in k2.items():
                    kn[s] = max(kn.get(s, 0), v)
        for u in ups:
            if u.sync_type != "semaphore":
                continue
            cum[u.ant_name] = cum.get(u.ant_name, 0) + u.update_value
            kn[u.ant_name] = max(kn.get(u.ant_name, 0), cum[u.ant_name])
            snaps.setdefault(u.ant_name, []).append((cum[u.ant_name], kn))
        streams[skey] = kn

    bad = [
        (i.name, type(i).__name__, [(w.ant_name, w.wait_value) for w in i.sync_info.on_wait])
        for i in insts
        if i.sync_info is not None
        and len(i.sync_info.on_wait) > 1
        and type(i).__name__ not in ("InstDrain",)
    ]
    if bad:
        print(f"WARNING: {len(bad)} instructions still multi-wait: {bad[:6]}")


def _build_nc():
    nc = bass.Bass()
    hsT = nc.declare_dram_parameter("hsT", [128, KC], F16, isOutput=False)
    swo2 = nc.declare_dram_parameter("swo2", [128, G], F32, isOutput=False)
    wqkv = nc.declare_dram_parameter("wqkv", [8, 128, 4 * 768], I8, isOutput=False)
    kT = nc.declare_dram_parameter("kT", [128, S], I8, isOutput=False)
    ksc = nc.declare_dram_parameter("ksc", [128, S // 128], F32, isOutput=False)
    vsc = nc.declare_dram_parameter("vsc", [128, S // 128], F16, isOutput=False)
    v8 = nc.declare_dram_parameter("v8", [S // (128 * VCH), 128, VCH * D], F8E3, isOutput=False)
    wo8 = nc.declare_dram_parameter("wo8", [8, 128, G * 512], I8, isOutput=False)
    out = nc.declare_dram_parameter("out", [1, HID], F32, isOutput=True)

    PS = bass.MemorySpace.PSUM
    with tile.TileContext(nc) as tc:
        with (
            tc.tile_pool(name="const", bufs=1) as cpool,
            tc.tile_pool(name="w8p", bufs=4) as w8_pool,
            tc.tile_pool(name="w16ap", bufs=4) as w16a_pool,
            tc.tile_pool(name="w16bp", bufs=4) as w16b_pool,
            tc.tile_pool(name="kp", bufs=4) as k_pool,
            tc.tile_pool(name="k16p", bufs=7) as k16_pool,
            tc.tile_pool(name="vp", bufs=6) as v_pool,
            tc.tile_pool(name="wo8p", bufs=4) as wo8_pool,
            tc.tile_pool(name="wop", bufs=4) as wo_pool,
            tc.tile_pool(name="sm", bufs=1) as sm,
        ):
            # ---- constants + tiny loads (sync FIFO head) ----
            hs_sb = cpool.tile([128, KC], F16)
            nc.sync.dma_start(out=hs_sb, in_=hsT[:, :])
            swo_sb = cpool.tile([128, G], F32)
            nc.sync.dma_start(out=swo_sb, in_=swo2[:, :])
            ebias_sb = cpool.tile([128, 1], F32)
            nc.vector.memset(ebias_sb, EXP_BIAS)
            ones_sb = cpool.tile([128, 1], F32)
            nc.vector.memset(ones_sb, 1.0)
            ones_row = cpool.tile([1, 128], F32)
            nc.vector.memset(ones_row, 1.0)
            warm_sb = cpool.tile([128, 512], F16)
            nc.vector.memset(warm_sb, 0.0)
            zero16_sb = cpool.tile([128, 1], F16)
            nc.vector.memset(zero16_sb, 0.0)

            qk_sb = sm.tile([128, G + 1], F16)
            vrow_sb = sm.tile([1, D], F32)
            ksc_sb = cpool.tile([128, S // 128], F32)
            nc.sync.dma_start(out=ksc_sb, in_=ksc[:, :])
            vsc_sb = cpool.tile([128, S // 128], F16)
            nc.sync.dma_start(out=vsc_sb, in_=vsc[:, :])
            # tiny DVE reads so later DVE ops inherit the scale-DMA waits
            # through the engine stream (keeps every op single-wait)
            touch_sb = sm.tile([1, 2], F32)
            nc.vector.tensor_copy(out=touch_sb[:, 0:1], in_=ksc_sb[0:1, 0:1])
            nc.vector.tensor_copy(out=touch_sb[:, 1:2], in_=vsc_sb[0:1, 0:1])

            # ---- q/k/v projections; wqkv arrives int8, upcast on DVE/ACT ----
            with tc.tile_pool(name="psqk", bufs=1, space=PS) as psqk_pool:
                # HAM warm-up: keep PE busy through its cold window while
                # the first weight tiles are still in flight
                ps_warm = psqk_pool.tile([128, 512], F32, tag="warm")
                for _ in range(10):
                    nc.tensor.matmul(
                        ps_warm[:, :], lhsT=warm_sb[:, 0:128], rhs=warm_sb,
                        start=True, stop=True,
                    )

                ps_qk = [
                    psqk_pool.tile([128, 1], F32, name=f"ps_qk{h}", tag=f"qk{h}")
                    for h in range(G + 1)
                ]
                ps_v = psqk_pool.tile([1, D], F32, tag="psv")
                # wqkv tile layout: 24 col-blocks of 128 = (c, h) with
                # block = 6c + h.  DVE upcasts blocks 0-15, ACT 16-23
                # (separate dest tiles so every proj MM waits one writer).
                proj_last = []  # last proj MM per tile (threads pool WARs)
                for t in range(8):
                    w8_sb = w8_pool.tile([128, 4 * 768], I8, tag="w8")
                    wd = nc.sync.dma_start(out=w8_sb, in_=wqkv[t])
                    if t >= 4:
                        add_dep_helper(wd.ins, proj_last[t - 4].ins, sync=True,
                                       reason="w8/w16 slots free before reuse")
                    w16a_sb = w16a_pool.tile([128, 2048], F16, tag="w16a")
                    w16b_sb = w16b_pool.tile([128, 1024], F16, tag="w16b")
                    # tensor_tensor (not copy/cast) stays off the shared DVE
                    # port so GPSIMD SWDGE descriptor-gen is never starved
                    z_bc = bass.AP(tensor=zero16_sb.tensor, offset=zero16_sb.offset,
                                   ap=[zero16_sb.ap[0], [0, 2048]])
                    nc.vector.tensor_add(out=w16a_sb, in0=w8_sb[:, 0:2048], in1=z_bc)
                    nc.scalar.copy(out=w16b_sb, in_=w8_sb[:, 2048:3072])

                    def wslice(c, h):
                        blk = 6 * c + h
                        if blk < 16:
                            return w16a_sb[:, blk * 128 : (blk + 1) * 128]
                        return w16b_sb[:, (blk - 16) * 128 : (blk - 15) * 128]

                    for c in range(4):
                        j = 4 * t + c
                        for h in range(G + 1):
                            nc.tensor.matmul(
                                ps_qk[h][:, :],
                                lhsT=wslice(c, h),
                                rhs=hs_sb[:, j : j + 1],
                                start=(j == 0),
                                stop=(j == KC - 1),
                            )
                        vmm = nc.tensor.matmul(
                            ps_v[:, :],
                            lhsT=hs_sb[:, j : j + 1],
                            rhs=wslice(c, 5),
                            start=(j == 0),
                            stop=(j == KC - 1),
                        )
                    proj_last.append(vmm)
                for h in range(G + 1):
                    nc.vector.tensor_copy(out=qk_sb[:, h : h + 1], in_=ps_qk[h])
                # 2^14 matches the vsc-folded PV accumulation; /CHS undoes
                # the hsT pre-scale
                nc.scalar.mul(out=vrow_sb, in_=ps_v, mul=16384.0 / CHS)

            with (
                tc.tile_pool(name="pssc", bufs=1, space=PS) as pssc_pool,
                tc.tile_pool(name="pspv", bufs=1, space=PS) as pspv_pool,
                tc.tile_pool(name="psms", bufs=1, space=PS) as psms_pool,
            ):
                # ---- current-token score row: s_curT[1, g] = k_cur . q_g ----
                # q and k both carry a CHS factor from hsT -> scale 1/CHS^2
                ps_scur = psms_pool.tile([1, G], F32, tag="ms")
                nc.tensor.matmul(
                    ps_scur[:, :], lhsT=qk_sb[:, G : G + 1], rhs=qk_sb[:, 0:G],
                    start=True, stop=True,
                )
                pcurf_sb = sm.tile([1, G], F32)
                nc.scalar.activation(
                    out=pcurf_sb, in_=ps_scur,
                    func=mybir.ActivationFunctionType.Exp, bias=ebias_sb[:1],
                    scale=float(RSQD / (CHS * CHS)),
                )

                # ---- scores over the cache: [s, g] layout, 2 PSUM banks ----
                ps_sc = [
                    pssc_pool.tile([128, 512], F32, name=f"ps_sc{b}", tag=f"sc{b}")
                    for b in range(2)
                ]
                probs_sb = [
                    sm.tile([128, 512], F16, name=f"probs{b}", tag=f"pr{b}")
                    for b in range(2)
                ]
                pprime_sb = [
                    sm.tile([128, 512], F16, name=f"pprime{b}", tag=f"pp{b}")
                    for b in range(2)
                ]
                dpart_sb = sm.tile([128, 2 * G], F32)
                kcpt = KTILE // 128  # score chunks per K tile
                score_last = []  # last score MM per tile (threads pool WARs)
                for co in range(S // KTILE):
                    k_sb = k_pool.tile([128, KTILE], I8, tag="kt")
                    kd = nc.sync.dma_start(
                        out=k_sb, in_=kT[:, co * KTILE : (co + 1) * KTILE]
                    )
                    if co == 7:
                        # the only k16-slot recycle; score_last[3] also covers
                        # the k_i8 slot WAR, keeping DMA + upcast single-wait
                        add_dep_helper(kd.ins, score_last[3].ins, sync=True,
                                       reason="k16 slot free before last kT")
                    k16_sb = k16_pool.tile([128, KTILE], F16, tag="k16")
                    # balanced upcast split: DVE 2x-accel takes 5 tiles, ACT
                    # 1x takes 3; tiles 0,1 stay on DVE so ACT tiles inherit
                    # the qk knowledge through PE stream history
                    if co in (2, 4, 6):
                        nc.scalar.copy(out=k16_sb, in_=k_sb)
                    else:
                        nc.vector.tensor_copy(out=k16_sb, in_=k_sb)
                    for ci in range(kcpt):
                        ch = co * kcpt + ci
                        b, col = ch // 128, (ch % 128) * 4
                        mm = nc.tensor.matmul(
                            ps_sc[b][:, col : col + 4],
                            lhsT=k16_sb[:, ci * 128 : (ci + 1) * 128],
                            rhs=qk_sb[:, 0:G],
                            start=True,
                            stop=True,
                        )
                    score_last.append(mm)
                    if (co + 1) * kcpt % 128 == 0:
                        b = ((co + 1) * kcpt - 1) // 128
                        # scores = raw_codes_dot * k_scale[s]  (per-s scale,
                        # broadcast over the 4 g columns)
                        kb = ksc_sb[:, b * 128 : (b + 1) * 128]
                        kb_bc = bass.AP(tensor=kb.tensor, offset=kb.offset,
                                        ap=[*kb.ap, [0, G]])
                        scraw = sm.tile([128, 512], F32, name=f"scraw{b}", tag="scr", bufs=2)
                        nc.vector.tensor_mul(
                            out=scraw.rearrange("p (c g) -> p c g", g=G),
                            in0=ps_sc[b].rearrange("p (c g) -> p c g", g=G),
                            in1=kb_bc,
                        )
                        nc.scalar.activation(
                            out=probs_sb[b], in_=scraw,
                            func=mybir.ActivationFunctionType.Exp, bias=ebias_sb,
                        )
                        # per-(partition, g) partials: reduce over the 128
                        # chunk-columns (stride 4) of the bank
                        nc.vector.reduce_sum(
                            out=dpart_sb[:, b * G : (b + 1) * G],
                            in_=probs_sb[b].rearrange("p (c g) -> p g c", g=G),
                            axis=mybir.AxisListType.X,
                        )
                        # fold v_scale[s] into the probabilities used by PV
                        vb = vsc_sb[:, b * 128 : (b + 1) * 128]
                        vb_bc = bass.AP(tensor=vb.tensor, offset=vb.offset,
                                        ap=[*vb.ap, [0, G]])
                        nc.vector.tensor_mul(
                            out=pprime_sb[b].rearrange("p (c g) -> p c g", g=G),
                            in0=probs_sb[b].rearrange("p (c g) -> p c g", g=G),
                            in1=vb_bc,
                        )

                # ---- denominator; rden broadcast; fold wo row scales ----
                ps_den = psms_pool.tile([1, 2 * G], F32, tag="ms")
                nc.tensor.matmul(
                    ps_den[:, :], lhsT=ones_sb, rhs=dpart_sb, start=True, stop=True,
                )
                den_sb = sm.tile([1, 2 * G], F32)
                nc.vector.tensor_copy(out=den_sb, in_=ps_den)
                dtot_sb = sm.tile([1, G], F32)
                nc.vector.tensor_add(
                    out=dtot_sb, in0=den_sb[:, 0:G], in1=den_sb[:, G : 2 * G]
                )
                nc.vector.tensor_add(out=dtot_sb, in0=dtot_sb, in1=pcurf_sb)
                rden_sb = sm.tile([1, G], F32)
                nc.vector.reciprocal(out=rden_sb, in_=dtot_sb)
                # broadcast rden across partitions on PE (ones outer product)
                ps_bc = psms_pool.tile([128, G], F32, tag="ms")
                nc.tensor.matmul(
                    ps_bc[:, :], lhsT=ones_row, rhs=rden_sb, start=True, stop=True
                )
                bc_sb = sm.tile([128, G], F32)
                # bc = rden[g] * wo_row_scale[p,g] * 2^-14
                nc.vector.tensor_mul(out=bc_sb, in0=ps_bc, in1=swo_sb)

                # ---- PV: outT[d, g] over all 256 chunks + current token ----
                v_dmas = []
                ps_pv = pspv_pool.tile([128, G], F32)
                for co in range(S // (128 * VCH)):
                    v_sb = v_pool.tile([128, VCH * D], F8E3, tag="vt")
                    vd = nc.sync.dma_start(out=v_sb, in_=v8[co])
                    v_dmas.append(vd)
                    for ci in range(VCH):
                        ch = co * VCH + ci
                        b, col = ch // 128, (ch % 128) * 4
                        nc.tensor.matmul(
                            ps_pv[:, :],
                            lhsT=v_sb[:, ci * D : (ci + 1) * D],
                            rhs=pprime_sb[b][:, col : col + 4],
                            start=(ch == 0),
                            stop=False,
                        )
                nc.tensor.matmul(
                    ps_pv[:, :], lhsT=vrow_sb, rhs=pcurf_sb, start=False, stop=True,
                )
                outn_sb = sm.tile([128, G], F16)
                nc.vector.tensor_mul(out=outn_sb, in0=ps_pv, in1=bc_sb)

            # ---- o_proj, pipelined per 512-col chunk behind the wo stream ----
            with tc.tile_pool(name="pso", bufs=2, space=PS) as pso_pool:
                ofin_sb = sm.tile([1, HID], F32)
                ofin_copies = []
                for n in range(8):
                    wo8_sb = wo8_pool.tile([128, G * 512], I8, tag="wo8")
                    wd = nc.sync.dma_start(out=wo8_sb, in_=wo8[n])
                    if n >= 2:
                        # thread the ps_on / wo16-slot recycle WARs through
                        # the wo DMA so everything downstream is single-wait
                        add_dep_helper(wd.ins, ofin_copies[n - 2].ins, sync=True,
                                       reason="ps_on bank free before wo lands")
                    wo_sb = wo_pool.tile([128, G * 512], F16, tag="wo")
                    if n in (1, 5):
                        nc.scalar.copy(out=wo_sb, in_=wo8_sb)
                    else:
                        nc.vector.tensor_copy(out=wo_sb, in_=wo8_sb)
                    ps_on = pso_pool.tile([1, 512], F32, tag="on")
                    for g in range(G):
                        nc.tensor.matmul(
                            ps_on[:, :],
                            lhsT=outn_sb[:, g : g + 1],
                            rhs=wo_sb[:, g * 512 : (g + 1) * 512],
                            start=(g == 0),
                            stop=(g == G - 1),
                        )
                    ofin_copies.append(
                        nc.scalar.copy(out=ofin_sb[:, n * 512 : (n + 1) * 512], in_=ps_on)
                    )
            nc.gpsimd.dma_start(out=out[:, :], in_=ofin_sb)

    _reduce_dma_waits(nc)
    return nc


def _rope_fold(W, nheads, cos, sin, scale=1.0):
    """Fold RoPE rotation (and an optional scalar) into projection weights."""
    W = W.reshape(HID, nheads, D).astype(np.float32)
    half = D // 2
    Wr = np.empty_like(W)
    Wr[:, :, :half] = cos[:half] * W[:, :, :half] - sin[:half] * W[:, :, half:]
    Wr[:, :, half:] = cos[half:] * W[:, :, half:] + sin[half:] * W[:, :, :half]
    return (Wr * np.float32(scale)).reshape(HID, nheads * D)


def _prep_inputs(hidden_states, k_qx, k_scale, v_qx, v_scale, cos, sin, Wq, Wk, Wv, Wo):
    f16 = np.float16
    f8 = ml_dtypes.float8_e3m4
    hs = np.ascontiguousarray(hidden_states.reshape(HID)).astype(np.float32)
    cos = cos.astype(np.float32)
    sin = sin.astype(np.float32)
    Wq_f = _rope_fold(Wq, H, cos, sin)  # 1/sqrt(D) lives in ksc / scur scale
    Wk_f = _rope_fold(Wk, HKV, cos, sin)

    in_maps = []
    for c in range(NCORES):
        qcols = slice(G * c * D, G * (c + 1) * D)
        kvcols = slice(c * D, (c + 1) * D)
        Wsl = np.concatenate(
            [Wq_f[:, qcols], Wk_f[:, kvcols], Wv[:, kvcols].astype(np.float32)],
            axis=1,
        )  # [HID, 768]
        srow = np.maximum(np.abs(Wsl).max(axis=1) / 127.0, 1e-12)
        w8 = np.clip(np.rint(Wsl / srow[:, None]), -127, 127).astype(np.int8)
        wqkv = np.ascontiguousarray(
            w8.reshape(8, 4, 128, 768).transpose(0, 2, 1, 3)
        ).reshape(8, 128, 4 * 768)
        hsT = np.ascontiguousarray(
            (hs * srow * CHS).reshape(KC, 128).T
        ).astype(f16)

        kT = np.ascontiguousarray(k_qx[:, c, :].astype(np.int8).T)
        vcodes = v_qx[:, c, :].astype(np.float32)
        vf8 = (vcodes / F8S).astype(f8)
        v8a = np.ascontiguousarray(
            vf8.reshape(S // (128 * VCH), VCH, 128, D).transpose(0, 2, 1, 3)
        ).reshape(S // (128 * VCH), 128, VCH * D)
        # per-token LSQ refit of v_scale against the fp8-rounded codes
        vhat = vf8.astype(np.float32) * F8S
        adj = (vcodes * vhat).sum(1) / np.maximum((vhat * vhat).sum(1), 1e-9)
        ksc = np.ascontiguousarray(
            (k_scale[:, c, 0].astype(np.float32) * RSQD / CHS)
            .reshape(S // 128, 128).T
        ).astype(np.float32)
        vsc = np.ascontiguousarray(
            (v_scale[:, c, 0].astype(np.float32) * adj * F8S * 16384.0)
            .reshape(S // 128, 128).T
        ).astype(f16)

        Wol = Wo[G * c * D : G * (c + 1) * D, :].astype(np.float32)  # [512, HID]
        srow_o = np.maximum(np.abs(Wol).max(axis=1) / 127.0, 1e-12)
        wo8f = np.clip(np.rint(Wol / srow_o[:, None]), -127, 127).astype(np.int8)
        wo8 = np.ascontiguousarray(
            wo8f.reshape(G, 128, 8, 512).transpose(2, 1, 0, 3)
        ).reshape(8, 128, G * 512)
        swo2 = np.ascontiguousarray(
            (srow_o / 16384.0).reshape(G, 128).T
        ).astype(np.float32)

        in_maps.append(
            {"hsT": hsT, "swo2": swo2, "wqkv": wqkv, "kT": kT, "v8": v8a,
             "wo8": wo8, "ksc": ksc, "vsc": vsc}
        )
    return in_maps


def _run(in_maps, trace=False, **kw):
    if "nc" not in _CACHE:
        _CACHE["nc"] = _build_nc()
    return run_bass_kernel_spmd(
        _CACHE["nc"], in_maps, core_ids=list(range(NCORES)), trace=trace, **kw
    )


def kernel(hidden_states, k_qx, k_scale, v_qx, v_scale, cos, sin, Wq, Wk, Wv, Wo):
    in_maps = _prep_inputs(
        hidden_states, k_qx, k_scale, v_qx, v_scale, cos, sin, Wq, Wk, Wv, Wo
    )
    res = _run(in_maps)
    out = np.zeros((1, 1, HID), np.float32)
    for r in res.results:
        out += r["out"].reshape(1, 1, HID)
    return out


# revision 32
# speedup vs baseline: 1.2310x; 1.1612x over previous
"""Trainium2 Bass kernel for Llama SmartKV decode attention (GQA, q_len=1).

Sharding: tensor-parallel over KV heads — core c owns kv head c and its
GQA group of 4 query heads (slices of Wq/Wk/Wv/Wo), plus that head's
quantized KV cache. Each core computes its partial o_proj output; the
host sums the 8 partials (the all-reduce).

Byte-budget design (per core, the binding resources):
  - Projection weights are int8 in DRAM with per-ROW scales folded into
    the host-prepped hsT (wqkv) and the PV-descale vector (wo), so no
    on-chip scale corrections are needed.  wqkv is DMA'd raw (HWDGE) and
    upcast to fp16 on DVE/ACT (engine ports, not the DMA fabric); wo is
    SWDGE-cast late when the fabric is idle.
  - KV cache codes are stored as fp8e3 (E3M4) = codes/16 (exact range
    +-7.94 within E3M4's +-15.5), read raw over HWDGE with no cast, and
    fed to the PE as the fp8 stationary operand (halves LDWEIGHTS time).
    The x16 is folded into k_scale/v_scale (stored fp16).
  - One HWDGE FIFO orders the big streams (hsT, wqkv, kT, v8) with no
    inter-stream dep sems; wo streams last so o_proj pipelines per-chunk
    behind it, shrinking the tail to ~2-3us.
HBM ~13.9MB (the 358GB/s floor), SBUF-fabric ~15.9MB, PE ~36us.
"""

import os

os.environ.setdefault("BY_DEFAULT_DISABLE_SUBTILE_DEPS", "1")

import ml_dtypes
import numpy as np

import concourse.bass as bass
import concourse.mybir as mybir
import concourse.tile as tile
from concourse.bass_utils import run_bass_kernel_spmd
from concourse.tile_rust import add_dep_helper

H, HKV, D, HID, S = 32, 8, 128, 4096, 32768
G = H // HKV  # 4 query heads per core
NCORES = 8
KC = HID // 128  # 32 contraction chunks for projections
KTILE = 4096  # tokens per K-cache DMA tile
VCH = 32  # s-chunks per V-cache DMA tile
F16 = mybir.dt.float16
F8E3 = mybir.dt.float8e3
I8 = mybir.dt.int8
F32 = mybir.dt.float32
EXP_BIAS = -9.0  # exp(s + B): cancels in softmax, keeps fp16 in range
CHS = 2.0**6  # hs pre-scale: keeps hsT entries in fp16 normal range
F8S = 16.0  # fp8e3 cache codes are stored as codes/16
RSQD = 1.0 / np.sqrt(np.float32(D))  # score scale (NOT folded into Wq:
# folding it would shrink Wq 11x vs Wk/Wv and waste the shared per-row
# int8 levels; applied via ksc and the scur activation scale instead)

_CACHE = {}


def _reduce_dma_waits(nc):
    """Drop transitively-implied waits from instructions.

    The PSEUDO_DMA_DIRECT2D descriptor holds exactly one wait slot, but
    Tile's sem assignment is not transitively minimal (its optimize_sems
    pass is disabled), so pool-slot-recycling DMAs carry a redundant
    second wait: the WAW wait on the previous slot writer is already
    implied by the engine-reader wait.  We verify implication with a
    vector-clock walk over the scheduled program and delete only waits
    that are provably redundant.
    """
    import bass_rust as _br

    insts = []
    for f in nc.m.functions:
        for bb in f.blocks:
            insts.extend(bb.instructions)

    cum = {}  # sem name -> cumulative value so far in schedule order
    snaps = {}  # sem name -> list of (cumval, knowledge dict)
    streams = {}  # stream key -> knowledge dict (sem name -> value known >=)

    def know_at(sem, val):
        # knowledge of the producer that first brought `sem` to >= val
        for cv, kn in snaps.get(sem, ()):
            if cv >= val:
                return kn
        return None

    for inst in insts:
        si = inst.sync_info
        if si is None:
            continue
        waits = list(si.on_wait)
        ups = list(si.on_update)
        if ups and ups[0].ant_name.startswith(("DMASW", "DMAHW")):
            skey = ups[0].ant_name
        else:
            skey = f"eng:{inst.engine}"
        kn = dict(streams.get(skey, ()))

        imm = [
            w
            for w in waits
            if w.wait_mode == "sem-ge-imm" and w.sync_type == "semaphore"
        ]
        if len(imm) == len(waits) > 1:
            keep = []
            for w in waits:
                others = dict(kn)
                for w2 in waits:
                    if w2 is w:
                        continue
                    others[w2.ant_name] = max(
                        others.get(w2.ant_name, 0), w2.wait_value
                    )
                    k2 = know_at(w2.ant_name, w2.wait_value)
                    if k2:
                        for s, v in k2.items():
                            others[s] = max(others.get(s, 0), v)
                if others.get(w.ant_name, 0) >= w.wait_value:
                    continue  # implied: drop
                keep.append(w)
            if len(keep) < len(waits):
                inst.sync_info = _br.SyncInfo(on_wait=keep, on_update=ups)
                waits = keep

        # fold wait knowledge into this instruction's stream knowledge
        for w in waits:
            if w.wait_mode != "sem-ge-imm" or w.sync_type != "semaphore":
                continue
            kn[w.ant_name] = max(kn.get(w.ant_name, 0), w.wait_value)
            k2 = know_at(w.ant_name, w.wait_value)
            if k2:
                for s, v in k2.items():
                    kn[s] = max(kn.get(s, 0), v)
        for u in ups:
            if u.sync_type != "semaphore":
                continue
            cum[u.ant_name] = cum.get(u.ant_name, 0) + u.update_value
            kn[u.ant_name] = max(kn.get(u.ant_name, 0), cum[u.ant_name])
            snaps.setdefault(u.ant_name, []).append((cum[u.ant_name], kn))
        streams[skey] = kn

    bad = [
        (i.name, type(i).__name__, [(w.ant_name, w.wait_value) for w in i.sync_info.on_wait])
        for i in insts
        if i.sync_info is not None
        and len(i.sync_info.on_wait) > 1
        and type(i).__name__ not in ("InstDrain",)
    ]
    if bad:
        print(f"WARNING: {len(bad)} instructions still multi-wait: {bad[:6]}")


def _build_nc():
    nc = bass.Bass()
    hsT = nc.declare_dram_parameter("hsT", [128, KC], F16, isOutput=False)
    swo2 = nc.declare_dram_parameter("swo2", [128, G], F32, isOutput=False)
    wqkv = nc.declare_dram_parameter("wqkv", [8, 128, 4 * 768], I8, isOutput=False)
    kT = nc.declare_dram_parameter("kT", [128, S], I8, isOutput=False)
    ksc = nc.declare_dram_parameter("ksc", [128, S // 128], F32, isOutput=False)
    vsc = nc.declare_dram_parameter("vsc", [128, S // 128], F16, isOutput=False)
    v8 = nc.declare_dram_parameter("v8", [S // (128 * VCH), 128, VCH * D], F8E3, isOutput=False)
    wo8 = nc.declare_dram_parameter("wo8", [8, 128, G * 512], I8, isOutput=False)
    out = nc.declare_dram_parameter("out", [1, HID], F32, isOutput=True)

    PS = bass.MemorySpace.PSUM
    with tile.TileContext(nc) as tc:
        with (
            tc.tile_pool(name="const", bufs=1) as cpool,
            tc.tile_pool(name="w8p", bufs=4) as w8_pool,
            tc.tile_pool(name="w16ap", bufs=4) as w16a_pool,
            tc.tile_pool(name="w16bp", bufs=4) as w16b_pool,
            tc.tile_pool(name="kp", bufs=4) as k_pool,
            tc.tile_pool(name="k16p", bufs=7) as k16_pool,
            tc.tile_pool(name="vp", bufs=6) as v_pool,
            tc.tile_pool(name="wo8p", bufs=4) as wo8_pool,
            tc.tile_pool(name="wop", bufs=4) as wo_pool,
            tc.tile_pool(name="sm", bufs=1) as sm,
        ):
            # ---- constants + tiny loads (sync FIFO head) ----
            hs_sb = cpool.tile([128, KC], F16)
            nc.sync.dma_start(out=hs_sb, in_=hsT[:, :])
            swo_sb = cpool.tile([128, G], F32)
            nc.sync.dma_start(out=swo_sb, in_=swo2[:, :])
            ebias_sb = cpool.tile([128, 1], F32)
            nc.vector.memset(ebias_sb, EXP_BIAS)
            ones_sb = cpool.tile([128, 1], F32)
            nc.vector.memset(ones_sb, 1.0)
            ones_row = cpool.tile([1, 128], F32)
            nc.vector.memset(ones_row, 1.0)
            warm_sb = cpool.tile([128, 512], F16)
            nc.vector.memset(warm_sb, 0.0)
            zero16_sb = cpool.tile([128, 1], F16)
            nc.vector.memset(zero16_sb, 0.0)

            qk_sb = sm.tile([128, G + 1], F16)
            vrow_sb = sm.tile([1, D], F32)
            ksc_sb = cpool.tile([128, S // 128], F32)
            nc.sync.dma_start(out=ksc_sb, in_=ksc[:, :])
            vsc_sb = cpool.tile([128, S // 128], F16)
            nc.sync.dma_start(out=vsc_sb, in_=vsc[:, :])
            # tiny DVE reads so later DVE ops inherit the scale-DMA waits
            # through the engine stream (keeps every op single-wait)
            touch_sb = sm.tile([1, 2], F32)
            nc.vector.tensor_copy(out=touch_sb[:, 0:1], in_=ksc_sb[0:1, 0:1])
            nc.vector.tensor_copy(out=touch_sb[:, 1:2], in_=vsc_sb[0:1, 0:1])

            # ---- q/k/v projections; wqkv arrives int8, upcast on DVE/ACT ----
            with tc.tile_pool(name="psqk", bufs=1, space=PS) as psqk_pool:
                # HAM warm-up: keep PE busy through its cold window while
                # the first weight tiles are still in flight
                ps_warm = psqk_pool.tile([128, 512], F32, tag="warm")
                for _ in range(10):
                    nc.tensor.matmul(
                        ps_warm[:, :], lhsT=warm_sb[:, 0:128], rhs=warm_sb,
                        start=True, stop=True,
                    )

                ps_qk = [
                    psqk_pool.tile([128, 1], F32, name=f"ps_qk{h}", tag=f"qk{h}")
                    for h in range(G + 1)
                ]
                ps_v = psqk_pool.tile([1, D], F32, tag="psv")
                # wqkv tile layout: 24 col-blocks of 128 = (c, h) with
                # block = 6c + h.  DVE upcasts blocks 0-15, ACT 16-23
                # (separate dest tiles so every proj MM waits one writer).
                proj_last = []  # last proj MM per tile (threads pool WARs)
                for t in range(8):
                    w8_sb = w8_pool.tile([128, 4 * 768], I8, tag="w8")
                    wd = nc.sync.dma_start(out=w8_sb, in_=wqkv[t])
                    if t >= 4:
                        add_dep_helper(wd.ins, proj_last[t - 4].ins, sync=True,
                                       reason="w8/w16 slots free before reuse")
                    w16a_sb = w16a_pool.tile([128, 2048], F16, tag="w16a")
                    w16b_sb = w16b_pool.tile([128, 1024], F16, tag="w16b")
                    nc.vector.tensor_copy(out=w16a_sb, in_=w8_sb[:, 0:2048])
                    nc.scalar.copy(out=w16b_sb, in_=w8_sb[:, 2048:3072])

                    def wslice(c, h):
                        blk = 6 * c + h
                        if blk < 16:
                            return w16a_sb[:, blk * 128 : (blk + 1) * 128]
                        return w16b_sb[:, (blk - 16) * 128 : (blk - 15) * 128]

                    for c in range(4):
                        j = 4 * t + c
                        for h in range(G + 1):
                            nc.tensor.matmul(
                                ps_qk[h][:, :],
                                lhsT=wslice(c, h),
                                rhs=hs_sb[:, j : j + 1],
                                start=(j == 0),
                                stop=(j == KC - 1),
                            )
                        vmm = nc.tensor.matmul(
                            ps_v[:, :],
                            lhsT=hs_sb[:, j : j + 1],
                            rhs=wslice(c, 5),
                            start=(j == 0),
                            stop=(j == KC - 1),
                        )
                    proj_last.append(vmm)
                for h in range(G + 1):
                    nc.vector.tensor_copy(out=qk_sb[:, h : h + 1], in_=ps_qk[h])
                # 2^14 matches the vsc-folded PV accumulation; /CHS undoes
                # the hsT pre-scale
                nc.scalar.mul(out=vrow_sb, in_=ps_v, mul=16384.0 / CHS)

            with (
                tc.tile_pool(name="pssc", bufs=1, space=PS) as pssc_pool,
                tc.tile_pool(name="pspv", bufs=1, space=PS) as pspv_pool,
                tc.tile_pool(name="psms", bufs=1, space=PS) as psms_pool,
            ):
                # ---- current-token score row: s_curT[1, g] = k_cur . q_g ----
                # q and k both carry a CHS factor from hsT -> scale 1/CHS^2
                ps_scur = psms_pool.tile([1, G], F32, tag="ms")
                nc.tensor.matmul(
                    ps_scur[:, :], lhsT=qk_sb[:, G : G + 1], rhs=qk_sb[:, 0:G],
                    start=True, stop=True,
                )
                pcurf_sb = sm.tile([1, G], F32)
                nc.scalar.activation(
                    out=pcurf_sb, in_=ps_scur,
                    func=mybir.ActivationFunctionType.Exp, bias=ebias_sb[:1],
                    scale=float(RSQD / (CHS * CHS)),
                )

                # ---- scores over the cache: [s, g] layout, 2 PSUM banks ----
                ps_sc = [
                    pssc_pool.tile([128, 512], F32, name=f"ps_sc{b}", tag=f"sc{b}")
                    for b in range(2)
                ]
                probs_sb = [
                    sm.tile([128, 512], F16, name=f"probs{b}", tag=f"pr{b}")
                    for b in range(2)
                ]
                pprime_sb = [
                    sm.tile([128, 512], F16, name=f"pprime{b}", tag=f"pp{b}")
                    for b in range(2)
                ]
                dpart_sb = sm.tile([128, 2 * G], F32)
                kcpt = KTILE // 128  # score chunks per K tile
                score_last = []  # last score MM per tile (threads pool WARs)
                for co in range(S // KTILE):
                    k_sb = k_pool.tile([128, KTILE], I8, tag="kt")
                    kd = nc.sync.dma_start(
                        out=k_sb, in_=kT[:, co * KTILE : (co + 1) * KTILE]
                    )
                    if co == 7:
                        # the only k16-slot recycle; score_last[3] also covers
                        # the k_i8 slot WAR, keeping DMA + upcast single-wait
                        add_dep_helper(kd.ins, score_last[3].ins, sync=True,
                                       reason="k16 slot free before last kT")
                    k16_sb = k16_pool.tile([128, KTILE], F16, tag="k16")
                    # balanced upcast split: DVE 2x-accel takes 5 tiles, ACT
                    # 1x takes 3; tiles 0,1 stay on DVE so ACT tiles inherit
                    # the qk knowledge through PE stream history
                    if co in (2, 4, 6):
                        nc.scalar.copy(out=k16_sb, in_=k_sb)
                    else:
                        nc.vector.tensor_copy(out=k16_sb, in_=k_sb)
                    for ci in range(kcpt):
                        ch = co * kcpt + ci
                        b, col = ch // 128, (ch % 128) * 4
                        mm = nc.tensor.matmul(
                            ps_sc[b][:, col : col + 4],
                            lhsT=k16_sb[:, ci * 128 : (ci + 1) * 128],
                            rhs=qk_sb[:, 0:G],
                            start=True,
                            stop=True,
                        )
                    score_last.append(mm)
                    if (co + 1) * kcpt % 128 == 0:
                        b = ((co + 1) * kcpt - 1) // 128
                        # scores = raw_codes_dot * k_scale[s]  (per-s scale,
                        # broadcast over the 4 g columns)
                        kb = ksc_sb[:, b * 128 : (b + 1) * 128]
                        kb_bc = bass.AP(tensor=kb.tensor, offset=kb.offset,
                                        ap=[*kb.ap, [0, G]])
                        scraw = sm.tile([128, 512], F32, name=f"scraw{b}", tag="scr", bufs=2)
                        nc.vector.tensor_mul(
                            out=scraw.rearrange("p (c g) -> p c g", g=G),
                            in0=ps_sc[b].rearrange("p (c g) -> p c g", g=G),
                            in1=kb_bc,
                        )
                        nc.scalar.activation(
                            out=probs_sb[b], in_=scraw,
                            func=mybir.ActivationFunctionType.Exp, bias=ebias_sb,
                        )
                        # per-(partition, g) partials: reduce over the 128
                        # chunk-columns (stride 4) of the bank
                        nc.vector.reduce_sum(
                            out=dpart_sb[:, b * G : (b + 1) * G],
                            in_=probs_sb[b].rearrange("p (c g) -> p g c", g=G),
                            axis=mybir.AxisListType.X,
                        )
                        # fold v_scale[s] into the probabilities used by PV
                        vb = vsc_sb[:, b * 128 : (b + 1) * 128]
                        vb_bc = bass.AP(tensor=vb.tensor, offset=vb.offset,
                                        ap=[*vb.ap, [0, G]])
                        nc.vector.tensor_mul(
                            out=pprime_sb[b].rearrange("p (c g) -> p c g", g=G),
                            in0=probs_sb[b].rearrange("p (c g) -> p c g", g=G),
                            in1=vb_bc,
                        )

                # ---- denominator; rden broadcast; fold wo row scales ----
                ps_den = psms_pool.tile([1, 2 * G], F32, tag="ms")
                nc.tensor.matmul(
                    ps_den[:, :], lhsT=ones_sb, rhs=dpart_sb, start=True, stop=True,
                )
                den_sb = sm.tile([1, 2 * G], F32)
                nc.vector.tensor_copy(out=den_sb, in_=ps_den)
                dtot_sb = sm.tile([1, G], F32)
                nc.vector.tensor_add(
                    out=dtot_sb, in0=den_sb[:, 0:G], in1=den_sb[:, G : 2 * G]
                )
                nc.vector.tensor_add(out=dtot_sb, in0=dtot_sb, in1=pcurf_sb)
                rden_sb = sm.tile([1, G], F32)
                nc.vector.reciprocal(out=rden_sb, in_=dtot_sb)
                # broadcast rden across partitions on PE (ones outer product)
                ps_bc = psms_pool.tile([128, G], F32, tag="ms")
                nc.tensor.matmul(
                    ps_bc[:, :], lhsT=ones_row, rhs=rden_sb, start=True, stop=True
                )
                bc_sb = sm.tile([128, G], F32)
                # bc = rden[g] * wo_row_scale[p,g] * 2^-14
                nc.vector.tensor_mul(out=bc_sb, in0=ps_bc, in1=swo_sb)

                # ---- PV: outT[d, g] over all 256 chunks + current token ----
                v_dmas = []
                ps_pv = pspv_pool.tile([128, G], F32)
                for co in range(S // (128 * VCH)):
                    v_sb = v_pool.tile([128, VCH * D], F8E3, tag="vt")
                    vd = nc.sync.dma_start(out=v_sb, in_=v8[co])
                    v_dmas.append(vd)
                    for ci in range(VCH):
                        ch = co * VCH + ci
                        b, col = ch // 128, (ch % 128) * 4
                        nc.tensor.matmul(
                            ps_pv[:, :],
                            lhsT=v_sb[:, ci * D : (ci + 1) * D],
                            rhs=pprime_sb[b][:, col : col + 4],
                            start=(ch == 0),
                            stop=False,
                        )
                nc.tensor.matmul(
                    ps_pv[:, :], lhsT=vrow_sb, rhs=pcurf_sb, start=False, stop=True,
                )
                outn_sb = sm.tile([128, G], F16)
                nc.vector.tensor_mul(out=outn_sb, in0=ps_pv, in1=bc_sb)

            # ---- o_proj, pipelined per 512-col chunk behind the wo stream ----
            with tc.tile_pool(name="pso", bufs=2, space=PS) as pso_pool:
                ofin_sb = sm.tile([1, HID], F32)
                ofin_copies = []
                for n in range(8):
                    wo8_sb = wo8_pool.tile([128, G * 512], I8, tag="wo8")
                    wd = nc.sync.dma_start(out=wo8_sb, in_=wo8[n])
                    if n >= 2:
                        # thread the ps_on / wo16-slot recycle WARs through
                        # the wo DMA so everything downstream is single-wait
                        add_dep_helper(wd.ins, ofin_copies[n - 2].ins, sync=True,
                                       reason="ps_on bank free before wo lands")
                    wo_sb = wo_pool.tile([128, G * 512], F16, tag="wo")
                    if n in (1, 5):
                        nc.scalar.copy(out=wo_sb, in_=wo8_sb)
                    else:
                        nc.vector.tensor_copy(out=wo_sb, in_=wo8_sb)
                    ps_on = pso_pool.tile([1, 512], F32, tag="on")
                    for g in range(G):
                        nc.tensor.matmul(
                            ps_on[:, :],
                            lhsT=outn_sb[:, g : g + 1],
                            rhs=wo_sb[:, g * 512 : (g + 1) * 512],
                            start=(g == 0),
                            stop=(g == G - 1),
                        )
                    ofin_copies.append(
                        nc.scalar.copy(out=ofin_sb[:, n * 512 : (n + 1) * 512], in_=ps_on)
                    )
            nc.gpsimd.dma_start(out=out[:, :], in_=ofin_sb)

    _reduce_dma_waits(nc)
    return nc


def _rope_fold(W, nheads, cos, sin, scale=1.0):
    """Fold RoPE rotation (and an optional scalar) into projection weights."""
    W = W.reshape(HID, nheads, D).astype(np.float32)
    half = D // 2
    Wr = np.empty_like(W)
    Wr[:, :, :half] = cos[:half] * W[:, :, :half] - sin[:half] * W[:, :, half:]
    Wr[:, :, half:] = cos[half:] * W[:, :, half:] + sin[half:] * W[:, :, :half]
    return (Wr * np.float32(scale)).reshape(HID, nheads * D)


def _prep_inputs(hidden_states, k_qx, k_scale, v_qx, v_scale, cos, sin, Wq, Wk, Wv, Wo):
    f16 = np.float16
    f8 = ml_dtypes.float8_e3m4
    hs = np.ascontiguousarray(hidden_states.reshape(HID)).astype(np.float32)
    cos = cos.astype(np.float32)
    sin = sin.astype(np.float32)
    Wq_f = _rope_fold(Wq, H, cos, sin)  # 1/sqrt(D) lives in ksc / scur scale
    Wk_f = _rope_fold(Wk, HKV, cos, sin)

    in_maps = []
    for c in range(NCORES):
        qcols = slice(G * c * D, G * (c + 1) * D)
        kvcols = slice(c * D, (c + 1) * D)
        Wsl = np.concatenate(
            [Wq_f[:, qcols], Wk_f[:, kvcols], Wv[:, kvcols].astype(np.float32)],
            axis=1,
        )  # [HID, 768]
        srow = np.maximum(np.abs(Wsl).max(axis=1) / 127.0, 1e-12)
        w8 = np.clip(np.rint(Wsl / srow[:, None]), -127, 127).astype(np.int8)
        wqkv = np.ascontiguousarray(
            w8.reshape(8, 4, 128, 768).transpose(0, 2, 1, 3)
        ).reshape(8, 128, 4 * 768)
        hsT = np.ascontiguousarray(
            (hs * srow * CHS).reshape(KC, 128).T
        ).astype(f16)

        kT = np.ascontiguousarray(k_qx[:, c, :].astype(np.int8).T)
        vcodes = v_qx[:, c, :].astype(np.float32)
        vf8 = (vcodes / F8S).astype(f8)
        v8a = np.ascontiguousarray(
            vf8.reshape(S // (128 * VCH), VCH, 128, D).transpose(0, 2, 1, 3)
        ).reshape(S // (128 * VCH), 128, VCH * D)
        # per-token LSQ refit of v_scale against the fp8-rounded codes
        vhat = vf8.astype(np.float32) * F8S
        adj = (vcodes * vhat).sum(1) / np.maximum((vhat * vhat).sum(1), 1e-9)
        ksc = np.ascontiguousarray(
            (k_scale[:, c, 0].astype(np.float32) * RSQD / CHS)
            .reshape(S // 128, 128).T
        ).astype(np.float32)
        vsc = np.ascontiguousarray(
            (v_scale[:, c, 0].astype(np.float32) * adj * F8S * 16384.0)
            .reshape(S // 128, 128).T
        ).astype(f16)

        Wol = Wo[G * c * D : G * (c + 1) * D, :].astype(np.float32)  # [512, HID]
        srow_o = np.maximum(np.abs(Wol).max(axis=1) / 127.0, 1e-12)
        wo8f = np.clip(np.rint(Wol / srow_o[:, None]), -127, 127).astype(np.int8)
        wo8 = np.ascontiguousarray(
            wo8f.reshape(G, 128, 8, 512).transpose(2, 1, 0, 3)
        ).reshape(8, 128, G * 512)
        swo2 = np.ascontiguousarray(
            (srow_o / 16384.0).reshape(G, 128).T
        ).astype(np.float32)

        in_maps.append(
            {"hsT": hsT, "swo2": swo2, "wqkv": wqkv, "kT": kT, "v8": v8a,
             "wo8": wo8, "ksc": ksc, "vsc": vsc}
        )
    return in_maps


def _run(in_maps, trace=False, **kw):
    if "nc" not in _CACHE:
        _CACHE["nc"] = _build_nc()
    return run_bass_kernel_spmd(
        _CACHE["nc"], in_maps, core_ids=list(range(NCORES)), trace=trace, **kw
    )


def kernel(hidden_states, k_qx, k_scale, v_qx, v_scale, cos, sin, Wq, Wk, Wv, Wo):
    in_maps = _prep_inputs(
        hidden_states, k_qx, k_scale, v_qx, v_scale, cos, sin, Wq, Wk, Wv, Wo
    )
    res = _run(in_maps)
    out = np.zeros((1, 1, HID), np.float32)
    for r in res.results:
        out += r["out"].reshape(1, 1, HID)
    return out


# revision 37
# speedup vs baseline: 1.3669x; 1.1104x over previous
"""Trainium2 Bass kernel for Llama SmartKV decode attention (GQA, q_len=1).

Sharding: tensor-parallel over KV heads — core c owns kv head c and its
GQA group of 4 query heads (slices of Wq/Wk/Wv/Wo), plus that head's
quantized KV cache. Each core computes its partial o_proj output; the
host sums the 8 partials (the all-reduce).

Byte-budget design (per core, the binding resources):
  - Projection weights are int8 in DRAM with per-ROW scales folded into
    the host-prepped hsT (wqkv) and the PV-descale vector (wo), so no
    on-chip scale corrections are needed.  wqkv is DMA'd raw (HWDGE) and
    upcast to fp16 on DVE/ACT (engine ports, not the DMA fabric); wo is
    SWDGE-cast late when the fabric is idle.
  - KV cache codes are stored as fp8e3 (E3M4) = codes/16 (exact range
    +-7.94 within E3M4's +-15.5), read raw over HWDGE with no cast, and
    fed to the PE as the fp8 stationary operand (halves LDWEIGHTS time).
    The x16 is folded into k_scale/v_scale (stored fp16).
  - One HWDGE FIFO orders the big streams (hsT, wqkv, kT, v8) with no
    inter-stream dep sems; wo streams last so o_proj pipelines per-chunk
    behind it, shrinking the tail to ~2-3us.
HBM ~13.9MB (the 358GB/s floor), SBUF-fabric ~15.9MB, PE ~36us.
"""

import os

os.environ.setdefault("BY_DEFAULT_DISABLE_SUBTILE_DEPS", "1")

import ml_dtypes
import numpy as np

import concourse.bass as bass
import concourse.mybir as mybir
import concourse.tile as tile
from concourse.bass_utils import run_bass_kernel_spmd
from concourse.tile_rust import add_dep_helper

H, HKV, D, HID, S = 32, 8, 128, 4096, 32768
G = H // HKV  # 4 query heads per core
NCORES = 8
KC = HID // 128  # 32 contraction chunks for projections
KTILE = 4096  # tokens per K-cache DMA tile
VCH = 32  # s-chunks per V-cache DMA tile
F16 = mybir.dt.float16
F8E3 = mybir.dt.float8e3
I8 = mybir.dt.int8
F32 = mybir.dt.float32
EXP_BIAS = -9.0  # exp(s + B): cancels in softmax, keeps fp16 in range
CHS = 2.0**6  # hs pre-scale: keeps hsT entries in fp16 normal range
F8S = 16.0  # fp8e3 cache codes are stored as codes/16
RSQD = 1.0 / np.sqrt(np.float32(D))  # score scale (NOT folded into Wq:
# folding it would shrink Wq 11x vs Wk/Wv and waste the shared per-row
# int8 levels; applied via ksc and the scur activation scale instead)

_CACHE = {}


def _reduce_dma_waits(nc):
    """Drop transitively-implied waits from instructions.

    The PSEUDO_DMA_DIRECT2D descriptor holds exactly one wait slot, but
    Tile's sem assignment is not transitively minimal (its optimize_sems
    pass is disabled), so pool-slot-recycling DMAs carry a redundant
    second wait: the WAW wait on the previous slot writer is already
    implied by the engine-reader wait.  We verify implication with a
    vector-clock walk over the scheduled program and delete only waits
    that are provably redundant.
    """
    import bass_rust as _br

    insts = []
    for f in nc.m.functions:
        for bb in f.blocks:
            insts.extend(bb.instructions)

    cum = {}  # sem name -> cumulative value so far in schedule order
    snaps = {}  # sem name -> list of (cumval, knowledge dict)
    streams = {}  # stream key -> knowledge dict (sem name -> value known >=)

    def know_at(sem, val):
        # knowledge of the producer that first brought `sem` to >= val
        for cv, kn in snaps.get(sem, ()):
            if cv >= val:
                return kn
        return None

    for inst in insts:
        si = inst.sync_info
        if si is None:
            continue
        waits = list(si.on_wait)
        ups = list(si.on_update)
        if ups and ups[0].ant_name.startswith(("DMASW", "DMAHW")):
            skey = ups[0].ant_name
        else:
            skey = f"eng:{inst.engine}"
        kn = dict(streams.get(skey, ()))

        imm = [
            w
            for w in waits
            if w.wait_mode == "sem-ge-imm" and w.sync_type == "semaphore"
        ]
        if len(imm) == len(waits) > 1:
            keep = []
            for w in waits:
                others = dict(kn)
                for w2 in waits:
                    if w2 is w:
                        continue
                    others[w2.ant_name] = max(
                        others.get(w2.ant_name, 0), w2.wait_value
                    )
                    k2 = know_at(w2.ant_name, w2.wait_value)
                    if k2:
                        for s, v in k2.items():
                            others[s] = max(others.get(s, 0), v)
                if others.get(w.ant_name, 0) >= w.wait_value:
                    continue  # implied: drop
                keep.append(w)
            if len(keep) < len(waits):
                inst.sync_info = _br.SyncInfo(on_wait=keep, on_update=ups)
                waits = keep

        # fold wait knowledge into this instruction's stream knowledge
        for w in waits:
            if w.wait_mode != "sem-ge-imm" or w.sync_type != "semaphore":
                continue
            kn[w.ant_name] = max(kn.get(w.ant_name, 0), w.wait_value)
            k2 = know_at(w.ant_name, w.wait_value)
            if k2:
                for s, v in k2.items():
                    kn[s] = max(kn.get(s, 0), v)
        for u in ups:
            if u.sync_type != "semaphore":
                continue
            cum[u.ant_name] = cum.get(u.ant_name, 0) + u.update_value
            kn[u.ant_name] = max(kn.get(u.ant_name, 0), cum[u.ant_name])
            snaps.setdefault(u.ant_name, []).append((cum[u.ant_name], kn))
        streams[skey] = kn

    bad = [
        (i.name, type(i).__name__, [(w.ant_name, w.wait_value) for w in i.sync_info.on_wait])
        for i in insts
        if i.sync_info is not None
        and len(i.sync_info.on_wait) > 1
        and type(i).__name__ not in ("InstDrain",)
    ]
    if bad:
        print(f"WARNING: {len(bad)} instructions still multi-wait: {bad[:6]}")


def _build_nc():
    nc = bass.Bass()
    hsT = nc.declare_dram_parameter("hsT", [128, KC], F16, isOutput=False)
    swo2 = nc.declare_dram_parameter("swo2", [128, G], F32, isOutput=False)
    wqkv = nc.declare_dram_parameter("wqkv", [8, 128, 4 * 768], I8, isOutput=False)
    kT = nc.declare_dram_parameter("kT", [128, S], I8, isOutput=False)
    ksc = nc.declare_dram_parameter("ksc", [128, S // 128], F32, isOutput=False)
    vsc = nc.declare_dram_parameter("vsc", [128, S // 128], F16, isOutput=False)
    v8 = nc.declare_dram_parameter("v8", [S // (128 * VCH), 128, VCH * D], F8E3, isOutput=False)
    wo8 = nc.declare_dram_parameter("wo8", [8, 128, G * 512], I8, isOutput=False)
    out = nc.declare_dram_parameter("out", [1, HID], F32, isOutput=True)

    PS = bass.MemorySpace.PSUM
    with tile.TileContext(nc) as tc:
        with (
            tc.tile_pool(name="const", bufs=1) as cpool,
            tc.tile_pool(name="w8p", bufs=4) as w8_pool,
            tc.tile_pool(name="w16ap", bufs=4) as w16a_pool,
            tc.tile_pool(name="w16bp", bufs=4) as w16b_pool,
            tc.tile_pool(name="kp", bufs=4) as k_pool,
            tc.tile_pool(name="k16p", bufs=7) as k16_pool,
            tc.tile_pool(name="vp", bufs=6) as v_pool,
            tc.tile_pool(name="wop", bufs=4) as wo_pool,
            tc.tile_pool(name="sm", bufs=1) as sm,
        ):
            # ---- constants + tiny loads (sync FIFO head) ----
            hs_sb = cpool.tile([128, KC], F16)
            nc.sync.dma_start(out=hs_sb, in_=hsT[:, :])
            swo_sb = cpool.tile([128, G], F32)
            nc.sync.dma_start(out=swo_sb, in_=swo2[:, :])
            ebias_sb = cpool.tile([128, 1], F32)
            nc.vector.memset(ebias_sb, EXP_BIAS)
            ones_sb = cpool.tile([128, 1], F32)
            nc.vector.memset(ones_sb, 1.0)
            ones_row = cpool.tile([1, 128], F32)
            nc.vector.memset(ones_row, 1.0)
            warm_sb = cpool.tile([128, 512], F16)
            nc.vector.memset(warm_sb, 0.0)
            zero16_sb = cpool.tile([128, 1], F16)
            nc.vector.memset(zero16_sb, 0.0)

            qk_sb = sm.tile([128, G + 1], F16)
            vrow_sb = sm.tile([1, D], F32)
            ksc_sb = cpool.tile([128, S // 128], F32)
            vsc_sb = cpool.tile([128, S // 128], F16)

            # ---- q/k/v projections; wqkv arrives int8, upcast on DVE/ACT ----
            with tc.tile_pool(name="psqk", bufs=1, space=PS) as psqk_pool:
                # HAM warm-up: keep PE busy through its cold window while
                # the first weight tiles are still in flight
                ps_warm = psqk_pool.tile([128, 512], F32, tag="warm")
                for _ in range(8):
                    nc.tensor.matmul(
                        ps_warm[:, :], lhsT=warm_sb[:, 0:128], rhs=warm_sb,
                        start=True, stop=True,
                    )

                ps_qk = [
                    psqk_pool.tile([128, 1], F32, name=f"ps_qk{h}", tag=f"qk{h}")
                    for h in range(G + 1)
                ]
                ps_v = psqk_pool.tile([1, D], F32, tag="psv")
                # wqkv tile layout: 24 col-blocks of 128 = (c, h) with
                # block = 6c + h.  DVE upcasts blocks 0-15, ACT 16-23
                # (separate dest tiles so every proj MM waits one writer).
                proj_last = []  # last proj MM per tile (threads pool WARs)
                for t in range(8):
                    w8_sb = w8_pool.tile([128, 4 * 768], I8, tag="w8")
                    wd = nc.sync.dma_start(out=w8_sb, in_=wqkv[t])
                    if t >= 4:
                        add_dep_helper(wd.ins, proj_last[t - 4].ins, sync=True,
                                       reason="w8/w16 slots free before reuse")
                    w16a_sb = w16a_pool.tile([128, 2048], F16, tag="w16a")
                    w16b_sb = w16b_pool.tile([128, 1024], F16, tag="w16b")
                    nc.vector.tensor_copy(out=w16a_sb, in_=w8_sb[:, 0:2048])
                    nc.scalar.copy(out=w16b_sb, in_=w8_sb[:, 2048:3072])

                    def wslice(c, h):
                        blk = 6 * c + h
                        if blk < 16:
                            return w16a_sb[:, blk * 128 : (blk + 1) * 128]
                        return w16b_sb[:, (blk - 16) * 128 : (blk - 15) * 128]

                    for c in range(4):
                        j = 4 * t + c
                        for h in range(G + 1):
                            nc.tensor.matmul(
                                ps_qk[h][:, :],
                                lhsT=wslice(c, h),
                                rhs=hs_sb[:, j : j + 1],
                                start=(j == 0),
                                stop=(j == KC - 1),
                            )
                        vmm = nc.tensor.matmul(
                            ps_v[:, :],
                            lhsT=hs_sb[:, j : j + 1],
                            rhs=wslice(c, 5),
                            start=(j == 0),
                            stop=(j == KC - 1),
                        )
                    proj_last.append(vmm)
                # scales ride the sync FIFO after the weights (needed ~10us
                # later, keeps the weight stream at the front)
                nc.sync.dma_start(out=ksc_sb, in_=ksc[:, :])
                nc.sync.dma_start(out=vsc_sb, in_=vsc[:, :])
                # tiny DVE reads so later DVE ops inherit the scale-DMA waits
                # through the engine stream (keeps every op single-wait)
                touch_sb = sm.tile([1, 2], F32)
                nc.vector.tensor_copy(out=touch_sb[:, 0:1], in_=ksc_sb[0:1, 0:1])
                nc.vector.tensor_copy(out=touch_sb[:, 1:2], in_=vsc_sb[0:1, 0:1])
                for h in range(G + 1):
                    nc.vector.tensor_copy(out=qk_sb[:, h : h + 1], in_=ps_qk[h])
                # 2^14 matches the vsc-folded PV accumulation; /CHS undoes
                # the hsT pre-scale
                nc.scalar.mul(out=vrow_sb, in_=ps_v, mul=16384.0 / CHS)

            with (
                tc.tile_pool(name="pssc", bufs=1, space=PS) as pssc_pool,
                tc.tile_pool(name="pspv", bufs=1, space=PS) as pspv_pool,
                tc.tile_pool(name="psms", bufs=1, space=PS) as psms_pool,
            ):
                # ---- current-token score row: s_curT[1, g] = k_cur . q_g ----
                # q and k both carry a CHS factor from hsT -> scale 1/CHS^2
                ps_scur = psms_pool.tile([1, G], F32, tag="ms")
                nc.tensor.matmul(
                    ps_scur[:, :], lhsT=qk_sb[:, G : G + 1], rhs=qk_sb[:, 0:G],
                    start=True, stop=True,
                )
                pcurf_sb = sm.tile([1, G], F32)
                nc.scalar.activation(
                    out=pcurf_sb, in_=ps_scur,
                    func=mybir.ActivationFunctionType.Exp, bias=ebias_sb[:1],
                    scale=float(RSQD / (CHS * CHS)),
                )

                # ---- scores over the cache: [s, g] layout, 2 PSUM banks ----
                ps_sc = [
                    pssc_pool.tile([128, 512], F32, name=f"ps_sc{b}", tag=f"sc{b}")
                    for b in range(2)
                ]
                probs_sb = [
                    sm.tile([128, 512], F16, name=f"probs{b}", tag=f"pr{b}")
                    for b in range(2)
                ]
                pprime_sb = [
                    sm.tile([128, 512], F16, name=f"pprime{b}", tag=f"pp{b}")
                    for b in range(2)
                ]
                dpart_sb = sm.tile([128, 2 * G], F32)
                kcpt = KTILE // 128  # score chunks per K tile
                score_last = []  # last score MM per tile (threads pool WARs)
                for co in range(S // KTILE):
                    k_sb = k_pool.tile([128, KTILE], I8, tag="kt")
                    kd = nc.sync.dma_start(
                        out=k_sb, in_=kT[:, co * KTILE : (co + 1) * KTILE]
                    )
                    if co == 7:
                        # the only k16-slot recycle; score_last[3] also covers
                        # the k_i8 slot WAR, keeping DMA + upcast single-wait
                        add_dep_helper(kd.ins, score_last[3].ins, sync=True,
                                       reason="k16 slot free before last kT")
                    k16_sb = k16_pool.tile([128, KTILE], F16, tag="k16")
                    # balanced upcast split: DVE 2x-accel takes 5 tiles, ACT
                    # 1x takes 3; tiles 0,1 stay on DVE so ACT tiles inherit
                    # the qk knowledge through PE stream history
                    if co in (2, 4, 6):
                        nc.scalar.copy(out=k16_sb, in_=k_sb)
                    else:
                        nc.vector.tensor_copy(out=k16_sb, in_=k_sb)
                    for ci in range(kcpt):
                        ch = co * kcpt + ci
                        b, col = ch // 128, (ch % 128) * 4
                        mm = nc.tensor.matmul(
                            ps_sc[b][:, col : col + 4],
                            lhsT=k16_sb[:, ci * 128 : (ci + 1) * 128],
                            rhs=qk_sb[:, 0:G],
                            start=True,
                            stop=True,
                        )
                    score_last.append(mm)
                    if (co + 1) * kcpt % 128 == 0:
                        b = ((co + 1) * kcpt - 1) // 128
                        # scores = raw_codes_dot * k_scale[s]  (per-s scale,
                        # broadcast over the 4 g columns)
                        kb = ksc_sb[:, b * 128 : (b + 1) * 128]
                        kb_bc = bass.AP(tensor=kb.tensor, offset=kb.offset,
                                        ap=[*kb.ap, [0, G]])
                        scraw = sm.tile([128, 512], F32, name=f"scraw{b}", tag="scr", bufs=2)
                        nc.vector.tensor_mul(
                            out=scraw.rearrange("p (c g) -> p c g", g=G),
                            in0=ps_sc[b].rearrange("p (c g) -> p c g", g=G),
                            in1=kb_bc,
                        )
                        nc.scalar.activation(
                            out=probs_sb[b], in_=scraw,
                            func=mybir.ActivationFunctionType.Exp, bias=ebias_sb,
                        )
                        # per-(partition, g) partials: reduce over the 128
                        # chunk-columns (stride 4) of the bank
                        nc.vector.reduce_sum(
                            out=dpart_sb[:, b * G : (b + 1) * G],
                            in_=probs_sb[b].rearrange("p (c g) -> p g c", g=G),
                            axis=mybir.AxisListType.X,
                        )
                        # fold v_scale[s] into the probabilities used by PV
                        vb = vsc_sb[:, b * 128 : (b + 1) * 128]
                        vb_bc = bass.AP(tensor=vb.tensor, offset=vb.offset,
                                        ap=[*vb.ap, [0, G]])
                        nc.vector.tensor_mul(
                            out=pprime_sb[b].rearrange("p (c g) -> p c g", g=G),
                            in0=probs_sb[b].rearrange("p (c g) -> p c g", g=G),
                            in1=vb_bc,
                        )

                # ---- denominator; rden broadcast; fold wo row scales ----
                ps_den = psms_pool.tile([1, 2 * G], F32, tag="ms")
                nc.tensor.matmul(
                    ps_den[:, :], lhsT=ones_sb, rhs=dpart_sb, start=True, stop=True,
                )
                den_sb = sm.tile([1, 2 * G], F32)
                nc.vector.tensor_copy(out=den_sb, in_=ps_den)
                dtot_sb = sm.tile([1, G], F32)
                nc.vector.tensor_add(
                    out=dtot_sb, in0=den_sb[:, 0:G], in1=den_sb[:, G : 2 * G]
                )
                nc.vector.tensor_add(out=dtot_sb, in0=dtot_sb, in1=pcurf_sb)
                rden_sb = sm.tile([1, G], F32)
                nc.vector.reciprocal(out=rden_sb, in_=dtot_sb)
                # broadcast rden across partitions on PE (ones outer product)
                ps_bc = psms_pool.tile([128, G], F32, tag="ms")
                nc.tensor.matmul(
                    ps_bc[:, :], lhsT=ones_row, rhs=rden_sb, start=True, stop=True
                )
                bc_sb = sm.tile([128, G], F32)
                # bc = rden[g] * wo_row_scale[p,g] * 2^-14
                nc.vector.tensor_mul(out=bc_sb, in0=ps_bc, in1=swo_sb)

                # ---- PV: outT[d, g] over all 256 chunks + current token ----
                v_dmas = []
                ps_pv = pspv_pool.tile([128, G], F32)
                for co in range(S // (128 * VCH)):
                    v_sb = v_pool.tile([128, VCH * D], F8E3, tag="vt")
                    vd = nc.sync.dma_start(out=v_sb, in_=v8[co])
                    v_dmas.append(vd)
                    for ci in range(VCH):
                        ch = co * VCH + ci
                        b, col = ch // 128, (ch % 128) * 4
                        nc.tensor.matmul(
                            ps_pv[:, :],
                            lhsT=v_sb[:, ci * D : (ci + 1) * D],
                            rhs=pprime_sb[b][:, col : col + 4],
                            start=(ch == 0),
                            stop=False,
                        )
                nc.tensor.matmul(
                    ps_pv[:, :], lhsT=vrow_sb, rhs=pcurf_sb, start=False, stop=True,
                )
                outn_sb = sm.tile([128, G], F16)
                outn_op = nc.vector.tensor_mul(out=outn_sb, in0=ps_pv, in1=bc_sb)

            # ---- o_proj, pipelined per 512-col chunk behind the wo stream.
            # wo arrives SWDGE-cast (PE-ready fp16); ps_on gets all 8 free
            # PSUM banks so no recycle chain; the single wo[0] <- outn dep
            # makes every downstream wait transitively implied.
            with tc.tile_pool(name="pso", bufs=8, space=PS) as pso_pool:
                ofin_sb = sm.tile([1, HID], F32)
                for n in range(8):
                    wo_sb = wo_pool.tile([128, G * 512], F16, tag="wo")
                    wd = nc.gpsimd.dma_start(out=wo_sb, in_=wo8[n])
                    if n == 0:
                        add_dep_helper(wd.ins, outn_op.ins, sync=True,
                                       reason="o_proj gated on outn anyway")
                    ps_on = pso_pool.tile([1, 512], F32, tag="on")
                    for g in range(G):
                        nc.tensor.matmul(
                            ps_on[:, :],
                            lhsT=outn_sb[:, g : g + 1],
                            rhs=wo_sb[:, g * 512 : (g + 1) * 512],
                            start=(g == 0),
                            stop=(g == G - 1),
                        )
                    nc.scalar.copy(out=ofin_sb[:, n * 512 : (n + 1) * 512], in_=ps_on)
            nc.gpsimd.dma_start(out=out[:, :], in_=ofin_sb)

    _reduce_dma_waits(nc)
    return nc


def _rope_fold(W, nheads, cos, sin, scale=1.0):
    """Fold RoPE rotation (and an optional scalar) into projection weights."""
    W = W.reshape(HID, nheads, D).astype(np.float32)
    half = D // 2
    Wr = np.empty_like(W)
    Wr[:, :, :half] = cos[:half] * W[:, :, :half] - sin[:half] * W[:, :, half:]
    Wr[:, :, half:] = cos[half:] * W[:, :, half:] + sin[half:] * W[:, :, :half]
    return (Wr * np.float32(scale)).reshape(HID, nheads * D)


def _prep_inputs(hidden_states, k_qx, k_scale, v_qx, v_scale, cos, sin, Wq, Wk, Wv, Wo):
    f16 = np.float16
    f8 = ml_dtypes.float8_e3m4
    hs = np.ascontiguousarray(hidden_states.reshape(HID)).astype(np.float32)
    cos = cos.astype(np.float32)
    sin = sin.astype(np.float32)
    Wq_f = _rope_fold(Wq, H, cos, sin)  # 1/sqrt(D) lives in ksc / scur scale
    Wk_f = _rope_fold(Wk, HKV, cos, sin)

    in_maps = []
    for c in range(NCORES):
        qcols = slice(G * c * D, G * (c + 1) * D)
        kvcols = slice(c * D, (c + 1) * D)
        Wsl = np.concatenate(
            [Wq_f[:, qcols], Wk_f[:, kvcols], Wv[:, kvcols].astype(np.float32)],
            axis=1,
        )  # [HID, 768]
        srow = np.maximum(np.abs(Wsl).max(axis=1) / 127.0, 1e-12)
        w8 = np.clip(np.rint(Wsl / srow[:, None]), -127, 127).astype(np.int8)
        wqkv = np.ascontiguousarray(
            w8.reshape(8, 4, 128, 768).transpose(0, 2, 1, 3)
        ).reshape(8, 128, 4 * 768)
        hsT = np.ascontiguousarray(
            (hs * srow * CHS).reshape(KC, 128).T
        ).astype(f16)

        kT = np.ascontiguousarray(k_qx[:, c, :].astype(np.int8).T)
        vcodes = v_qx[:, c, :].astype(np.float32)
        vf8 = (vcodes / F8S).astype(f8)
        v8a = np.ascontiguousarray(
            vf8.reshape(S // (128 * VCH), VCH, 128, D).transpose(0, 2, 1, 3)
        ).reshape(S // (128 * VCH), 128, VCH * D)
        # per-token LSQ refit of v_scale against the fp8-rounded codes
        vhat = vf8.astype(np.float32) * F8S
        adj = (vcodes * vhat).sum(1) / np.maximum((vhat * vhat).sum(1), 1e-9)
        ksc = np.ascontiguousarray(
            (k_scale[:, c, 0].astype(np.float32) * RSQD / CHS)
            .reshape(S // 128, 128).T
        ).astype(np.float32)
        vsc = np.ascontiguousarray(
            (v_scale[:, c, 0].astype(np.float32) * adj * F8S * 16384.0)
            .reshape(S // 128, 128).T
        ).astype(f16)

        Wol = Wo[G * c * D : G * (c + 1) * D, :].astype(np.float32)  # [512, HID]
        srow_o = np.maximum(np.abs(Wol).max(axis=1) / 127.0, 1e-12)
        wo8f = np.clip(np.rint(Wol / srow_o[:, None]), -127, 127).astype(np.int8)
        wo8 = np.ascontiguousarray(
            wo8f.reshape(G, 128, 8, 512).transpose(2, 1, 0, 3)
        ).reshape(8, 128, G * 512)
        swo2 = np.ascontiguousarray(
            (srow_o / 16384.0).reshape(G, 128).T
        ).astype(np.float32)

        in_maps.append(
            {"hsT": hsT, "swo2": swo2, "wqkv": wqkv, "kT": kT, "v8": v8a,
             "wo8": wo8, "ksc": ksc, "vsc": vsc}
        )
    return in_maps


def _run(in_maps, trace=False, **kw):
    if "nc" not in _CACHE:
        _CACHE["nc"] = _build_nc()
    return run_bass_kernel_spmd(
        _CACHE["nc"], in_maps, core_ids=list(range(NCORES)), trace=trace, **kw
    )


def kernel(hidden_states, k_qx, k_scale, v_qx, v_scale, cos, sin, Wq, Wk, Wv, Wo):
    in_maps = _prep_inputs(
        hidden_states, k_qx, k_scale, v_qx, v_scale, cos, sin, Wq, Wk, Wv, Wo
    )
    res = _run(in_maps)
    out = np.zeros((1, 1, HID), np.float32)
    for r in res.results:
        out += r["out"].reshape(1, 1, HID)
    return out


# revision 40
# speedup vs baseline: 1.4522x; 1.0624x over previous
"""Trainium2 Bass kernel for Llama SmartKV decode attention (GQA, q_len=1).

Sharding: tensor-parallel over KV heads — core c owns kv head c and its
GQA group of 4 query heads (slices of Wq/Wk/Wv/Wo), plus that head's
quantized KV cache. Each core computes its partial o_proj output; the
host sums the 8 partials (the all-reduce).

Byte-budget design (per core, the binding resources):
  - Projection weights are int8 in DRAM with per-ROW scales folded into
    the host-prepped hsT (wqkv) and the PV-descale vector (wo), so no
    on-chip scale corrections are needed.  wqkv is DMA'd raw (HWDGE) and
    upcast to fp16 on DVE/ACT (engine ports, not the DMA fabric); wo is
    SWDGE-cast late when the fabric is idle.
  - KV cache codes are stored as fp8e3 (E3M4) = codes/16 (exact range
    +-7.94 within E3M4's +-15.5), read raw over HWDGE with no cast, and
    fed to the PE as the fp8 stationary operand (halves LDWEIGHTS time).
    The x16 is folded into k_scale/v_scale (stored fp16).
  - One HWDGE FIFO orders the big streams (hsT, wqkv, kT, v8) with no
    inter-stream dep sems; wo streams last so o_proj pipelines per-chunk
    behind it, shrinking the tail to ~2-3us.
HBM ~13.9MB (the 358GB/s floor), SBUF-fabric ~15.9MB, PE ~36us.
"""

import os

os.environ.setdefault("BY_DEFAULT_DISABLE_SUBTILE_DEPS", "1")

import ml_dtypes
import numpy as np

import concourse.bass as bass
import concourse.mybir as mybir
import concourse.tile as tile
from concourse.bass_utils import run_bass_kernel_spmd
from concourse.tile_rust import add_dep_helper

H, HKV, D, HID, S = 32, 8, 128, 4096, 32768
G = H // HKV  # 4 query heads per core
NCORES = 8
KC = HID // 128  # 32 contraction chunks for projections
KTILE = 4096  # tokens per K-cache DMA tile
VCH = 32  # s-chunks per V-cache DMA tile
F16 = mybir.dt.float16
F8E3 = mybir.dt.float8e3
I8 = mybir.dt.int8
F32 = mybir.dt.float32
EXP_BIAS = -9.0  # exp(s + B): cancels in softmax, keeps fp16 in range
CHS = 2.0**6  # hs pre-scale: keeps hsT entries in fp16 normal range
F8S = 16.0  # fp8e3 cache codes are stored as codes/16
RSQD = 1.0 / np.sqrt(np.float32(D))  # score scale (NOT folded into Wq:
# folding it would shrink Wq 11x vs Wk/Wv and waste the shared per-row
# int8 levels; applied via ksc and the scur activation scale instead)

_CACHE = {}


def _reduce_dma_waits(nc):
    """Drop transitively-implied waits from instructions.

    The PSEUDO_DMA_DIRECT2D descriptor holds exactly one wait slot, but
    Tile's sem assignment is not transitively minimal (its optimize_sems
    pass is disabled), so pool-slot-recycling DMAs carry a redundant
    second wait: the WAW wait on the previous slot writer is already
    implied by the engine-reader wait.  We verify implication with a
    vector-clock walk over the scheduled program and delete only waits
    that are provably redundant.
    """
    import bass_rust as _br

    insts = []
    for f in nc.m.functions:
        for bb in f.blocks:
            insts.extend(bb.instructions)

    cum = {}  # sem name -> cumulative value so far in schedule order
    snaps = {}  # sem name -> list of (cumval, knowledge dict)
    streams = {}  # stream key -> knowledge dict (sem name -> value known >=)

    def know_at(sem, val):
        # knowledge of the producer that first brought `sem` to >= val
        for cv, kn in snaps.get(sem, ()):
            if cv >= val:
                return kn
        return None

    for inst in insts:
        si = inst.sync_info
        if si is None:
            continue
        waits = list(si.on_wait)
        ups = list(si.on_update)
        if ups and ups[0].ant_name.startswith(("DMASW", "DMAHW")):
            skey = ups[0].ant_name
        else:
            skey = f"eng:{inst.engine}"
        kn = dict(streams.get(skey, ()))

        imm = [
            w
            for w in waits
            if w.wait_mode == "sem-ge-imm" and w.sync_type == "semaphore"
        ]
        if len(imm) == len(waits) > 1:
            keep = []
            for w in waits:
                others = dict(kn)
                for w2 in waits:
                    if w2 is w:
                        continue
                    others[w2.ant_name] = max(
                        others.get(w2.ant_name, 0), w2.wait_value
                    )
                    k2 = know_at(w2.ant_name, w2.wait_value)
                    if k2:
                        for s, v in k2.items():
                            others[s] = max(others.get(s, 0), v)
                if others.get(w.ant_name, 0) >= w.wait_value:
                    continue  # implied: drop
                keep.append(w)
            if len(keep) < len(waits):
                inst.sync_info = _br.SyncInfo(on_wait=keep, on_update=ups)
                waits = keep

        # fold wait knowledge into this instruction's stream knowledge
        for w in waits:
            if w.wait_mode != "sem-ge-imm" or w.sync_type != "semaphore":
                continue
            kn[w.ant_name] = max(kn.get(w.ant_name, 0), w.wait_value)
            k2 = know_at(w.ant_name, w.wait_value)
            if k2:
                for s, v in k2.items():
                    kn[s] = max(kn.get(s, 0), v)
        for u in ups:
            if u.sync_type != "semaphore":
                continue
            cum[u.ant_name] = cum.get(u.ant_name, 0) + u.update_value
            kn[u.ant_name] = max(kn.get(u.ant_name, 0), cum[u.ant_name])
            snaps.setdefault(u.ant_name, []).append((cum[u.ant_name], kn))
        streams[skey] = kn

    bad = [
        (i.name, type(i).__name__, [(w.ant_name, w.wait_value) for w in i.sync_info.on_wait])
        for i in insts
        if i.sync_info is not None
        and len(i.sync_info.on_wait) > 1
        and type(i).__name__ not in ("InstDrain",)
    ]
    if bad:
        print(f"WARNING: {len(bad)} instructions still multi-wait: {bad[:6]}")


def _build_nc():
    nc = bass.Bass()
    hsT = nc.declare_dram_parameter("hsT", [128, KC], F16, isOutput=False)
    swo2 = nc.declare_dram_parameter("swo2", [128, G], F32, isOutput=False)
    wqkv = nc.declare_dram_parameter("wqkv", [8, 128, 4 * 768], I8, isOutput=False)
    kT = nc.declare_dram_parameter("kT", [128, S], I8, isOutput=False)
    ksc = nc.declare_dram_parameter("ksc", [128, S // 128], F32, isOutput=False)
    vsc = nc.declare_dram_parameter("vsc", [128, S // 128], F16, isOutput=False)
    v8 = nc.declare_dram_parameter("v8", [S // (128 * VCH), 128, VCH * D], F8E3, isOutput=False)
    wo8 = nc.declare_dram_parameter("wo8", [8, 128, G * 512], I8, isOutput=False)
    out = nc.declare_dram_parameter("out", [1, HID], F32, isOutput=True)

    PS = bass.MemorySpace.PSUM
    with tile.TileContext(nc) as tc:
        with (
            tc.tile_pool(name="const", bufs=1) as cpool,
            tc.tile_pool(name="w8p", bufs=4) as w8_pool,
            tc.tile_pool(name="w16ap", bufs=4) as w16a_pool,
            tc.tile_pool(name="w16bp", bufs=4) as w16b_pool,
            tc.tile_pool(name="kp", bufs=4) as k_pool,
            tc.tile_pool(name="k16p", bufs=14) as k16_pool,
            tc.tile_pool(name="vp", bufs=7) as v_pool,
            tc.tile_pool(name="wo8p", bufs=4) as wo8_pool,
            tc.tile_pool(name="wop", bufs=8) as wo_pool,
            tc.tile_pool(name="sm", bufs=1) as sm,
        ):
            # ---- constants + tiny loads (sync FIFO head) ----
            hs_sb = cpool.tile([128, KC], F16)
            nc.sync.dma_start(out=hs_sb, in_=hsT[:, :])
            swo_sb = cpool.tile([128, G], F32)
            nc.sync.dma_start(out=swo_sb, in_=swo2[:, :])
            ebias_sb = cpool.tile([128, 1], F32)
            nc.vector.memset(ebias_sb, EXP_BIAS)
            ones_sb = cpool.tile([128, 1], F32)
            nc.vector.memset(ones_sb, 1.0)
            ones_row = cpool.tile([1, 128], F32)
            nc.vector.memset(ones_row, 1.0)
            warm_sb = cpool.tile([128, 512], F16)
            nc.vector.memset(warm_sb, 0.0)
            zero16_sb = cpool.tile([128, 1], F16)
            nc.vector.memset(zero16_sb, 0.0)

            qk_sb = sm.tile([128, G + 1], F16)
            vrow_sb = sm.tile([1, D], F32)
            ksc_sb = cpool.tile([128, S // 128], F32)
            vsc_sb = cpool.tile([128, S // 128], F16)

            # ---- q/k/v projections; wqkv arrives int8, upcast on DVE/ACT ----
            with tc.tile_pool(name="psqk", bufs=1, space=PS) as psqk_pool:
                # HAM warm-up: keep PE busy through its cold window while
                # the first weight tiles are still in flight
                ps_warm = psqk_pool.tile([128, 512], F32, tag="warm")
                for _ in range(8):
                    nc.tensor.matmul(
                        ps_warm[:, :], lhsT=warm_sb[:, 0:128], rhs=warm_sb,
                        start=True, stop=True,
                    )

                ps_qk = [
                    psqk_pool.tile([128, 1], F32, name=f"ps_qk{h}", tag=f"qk{h}")
                    for h in range(G + 1)
                ]
                ps_v = psqk_pool.tile([1, D], F32, tag="psv")
                # wqkv tile layout: 24 col-blocks of 128 = (c, h) with
                # block = 6c + h.  DVE upcasts blocks 0-15, ACT 16-23
                # (separate dest tiles so every proj MM waits one writer).
                proj_last = []  # last proj MM per tile (threads pool WARs)
                for t in range(8):
                    w8_sb = w8_pool.tile([128, 4 * 768], I8, tag="w8")
                    wd = nc.sync.dma_start(out=w8_sb, in_=wqkv[t])
                    if t >= 4:
                        add_dep_helper(wd.ins, proj_last[t - 4].ins, sync=True,
                                       reason="w8/w16 slots free before reuse")
                    w16a_sb = w16a_pool.tile([128, 2048], F16, tag="w16a")
                    w16b_sb = w16b_pool.tile([128, 1024], F16, tag="w16b")
                    nc.vector.tensor_copy(out=w16a_sb, in_=w8_sb[:, 0:2048])
                    nc.scalar.copy(out=w16b_sb, in_=w8_sb[:, 2048:3072])

                    def wslice(c, h):
                        blk = 6 * c + h
                        if blk < 16:
                            return w16a_sb[:, blk * 128 : (blk + 1) * 128]
                        return w16b_sb[:, (blk - 16) * 128 : (blk - 15) * 128]

                    for c in range(4):
                        j = 4 * t + c
                        for h in range(G + 1):
                            nc.tensor.matmul(
                                ps_qk[h][:, :],
                                lhsT=wslice(c, h),
                                rhs=hs_sb[:, j : j + 1],
                                start=(j == 0),
                                stop=(j == KC - 1),
                            )
                        vmm = nc.tensor.matmul(
                            ps_v[:, :],
                            lhsT=hs_sb[:, j : j + 1],
                            rhs=wslice(c, 5),
                            start=(j == 0),
                            stop=(j == KC - 1),
                        )
                    proj_last.append(vmm)
                # scales ride the sync FIFO after the weights (needed ~10us
                # later, keeps the weight stream at the front)
                nc.sync.dma_start(out=ksc_sb, in_=ksc[:, :])
                nc.sync.dma_start(out=vsc_sb, in_=vsc[:, :])
                # tiny DVE reads so later DVE ops inherit the scale-DMA waits
                # through the engine stream (keeps every op single-wait)
                touch_sb = sm.tile([1, 2], F32)
                nc.vector.tensor_copy(out=touch_sb[:, 0:1], in_=ksc_sb[0:1, 0:1])
                nc.vector.tensor_copy(out=touch_sb[:, 1:2], in_=vsc_sb[0:1, 0:1])
                for h in range(G + 1):
                    nc.vector.tensor_copy(out=qk_sb[:, h : h + 1], in_=ps_qk[h])
                # 2^14 matches the vsc-folded PV accumulation; /CHS undoes
                # the hsT pre-scale
                nc.scalar.mul(out=vrow_sb, in_=ps_v, mul=16384.0 / CHS)

            with (
                tc.tile_pool(name="pssc", bufs=1, space=PS) as pssc_pool,
                tc.tile_pool(name="pspv", bufs=1, space=PS) as pspv_pool,
                tc.tile_pool(name="psms", bufs=1, space=PS) as psms_pool,
            ):
                # ---- current-token score row: s_curT[1, g] = k_cur . q_g ----
                # q and k both carry a CHS factor from hsT -> scale 1/CHS^2
                ps_scur = psms_pool.tile([1, G], F32, tag="ms")
                nc.tensor.matmul(
                    ps_scur[:, :], lhsT=qk_sb[:, G : G + 1], rhs=qk_sb[:, 0:G],
                    start=True, stop=True,
                )
                pcurf_sb = sm.tile([1, G], F32)
                nc.scalar.activation(
                    out=pcurf_sb, in_=ps_scur,
                    func=mybir.ActivationFunctionType.Exp, bias=ebias_sb[:1],
                    scale=float(RSQD / (CHS * CHS)),
                )

                # ---- scores over the cache: [s, g] layout, 2 PSUM banks ----
                ps_sc = [
                    pssc_pool.tile([128, 512], F32, name=f"ps_sc{b}", tag=f"sc{b}")
                    for b in range(2)
                ]
                probs_sb = [
                    sm.tile([128, 512], F16, name=f"probs{b}", tag=f"pr{b}")
                    for b in range(2)
                ]
                pprime_sb = [
                    sm.tile([128, 512], F16, name=f"pprime{b}", tag=f"pp{b}")
                    for b in range(2)
                ]
                dpart_sb = sm.tile([128, 2 * G], F32)
                kcpt = KTILE // 128  # score chunks per K tile
                score_last = []  # last score MM per tile (threads pool WARs)
                for co in range(S // KTILE):
                    k_sb = k_pool.tile([128, KTILE], I8, tag="kt")
                    kd = nc.sync.dma_start(
                        out=k_sb, in_=kT[:, co * KTILE : (co + 1) * KTILE]
                    )
                    if co == 7:
                        # the only k16-slot recycle; score_last[3] also covers
                        # the k_i8 slot WAR, keeping DMA + upcast single-wait
                        add_dep_helper(kd.ins, score_last[3].ins, sync=True,
                                       reason="k16 slot free before last kT")
                    # upcast in two half-tiles: bounds the DVE/ACT occupancy
                    # per op (so the critical qk copies aren't starved) and
                    # lets scores chase each half.  Both halves of a tile on
                    # one engine (single-wait WARs); DVE 2x-accel takes 5
                    # tiles, ACT 1x takes 3; tiles 0,1 stay on DVE so ACT
                    # tiles inherit the qk knowledge through PE history.
                    halves = []
                    for hh in range(2):
                        k16_sb = k16_pool.tile([128, KTILE // 2], F16, tag="k16")
                        src = k_sb[:, hh * (KTILE // 2) : (hh + 1) * (KTILE // 2)]
                        if co in (2, 4, 6):
                            nc.scalar.copy(out=k16_sb, in_=src)
                        else:
                            nc.vector.tensor_copy(out=k16_sb, in_=src)
                        halves.append(k16_sb)
                    hcpt = kcpt // 2
                    for ci in range(kcpt):
                        ch = co * kcpt + ci
                        b, col = ch // 128, (ch % 128) * 4
                        k16_sb = halves[ci // hcpt]
                        mm = nc.tensor.matmul(
                            ps_sc[b][:, col : col + 4],
                            lhsT=k16_sb[:, (ci % hcpt) * 128 : (ci % hcpt + 1) * 128],
                            rhs=qk_sb[:, 0:G],
                            start=True,
                            stop=True,
                        )
                    score_last.append(mm)
                    if (co + 1) * kcpt % 128 == 0:
                        b = ((co + 1) * kcpt - 1) // 128
                        # scores = raw_codes_dot * k_scale[s]  (per-s scale,
                        # broadcast over the 4 g columns)
                        kb = ksc_sb[:, b * 128 : (b + 1) * 128]
                        kb_bc = bass.AP(tensor=kb.tensor, offset=kb.offset,
                                        ap=[*kb.ap, [0, G]])
                        scraw = sm.tile([128, 512], F32, name=f"scraw{b}", tag="scr", bufs=2)
                        nc.vector.tensor_mul(
                            out=scraw.rearrange("p (c g) -> p c g", g=G),
                            in0=ps_sc[b].rearrange("p (c g) -> p c g", g=G),
                            in1=kb_bc,
                        )
                        nc.scalar.activation(
                            out=probs_sb[b], in_=scraw,
                            func=mybir.ActivationFunctionType.Exp, bias=ebias_sb,
                        )
                        # per-(partition, g) partials: reduce over the 128
                        # chunk-columns (stride 4) of the bank
                        nc.vector.reduce_sum(
                            out=dpart_sb[:, b * G : (b + 1) * G],
                            in_=probs_sb[b].rearrange("p (c g) -> p g c", g=G),
                            axis=mybir.AxisListType.X,
                        )
                        # fold v_scale[s] into the probabilities used by PV
                        vb = vsc_sb[:, b * 128 : (b + 1) * 128]
                        vb_bc = bass.AP(tensor=vb.tensor, offset=vb.offset,
                                        ap=[*vb.ap, [0, G]])
                        nc.vector.tensor_mul(
                            out=pprime_sb[b].rearrange("p (c g) -> p c g", g=G),
                            in0=probs_sb[b].rearrange("p (c g) -> p c g", g=G),
                            in1=vb_bc,
                        )

                # ---- denominator; rden broadcast; fold wo row scales ----
                ps_den = psms_pool.tile([1, 2 * G], F32, tag="ms")
                nc.tensor.matmul(
                    ps_den[:, :], lhsT=ones_sb, rhs=dpart_sb, start=True, stop=True,
                )
                den_sb = sm.tile([1, 2 * G], F32)
                nc.vector.tensor_copy(out=den_sb, in_=ps_den)
                dtot_sb = sm.tile([1, G], F32)
                nc.vector.tensor_add(
                    out=dtot_sb, in0=den_sb[:, 0:G], in1=den_sb[:, G : 2 * G]
                )
                nc.vector.tensor_add(out=dtot_sb, in0=dtot_sb, in1=pcurf_sb)
                rden_sb = sm.tile([1, G], F32)
                nc.vector.reciprocal(out=rden_sb, in_=dtot_sb)
                # broadcast rden across partitions on PE (ones outer product)
                ps_bc = psms_pool.tile([128, G], F32, tag="ms")
                nc.tensor.matmul(
                    ps_bc[:, :], lhsT=ones_row, rhs=rden_sb, start=True, stop=True
                )
                bc_sb = sm.tile([128, G], F32)
                # bc = rden[g] * wo_row_scale[p,g] * 2^-14
                nc.vector.tensor_mul(out=bc_sb, in0=ps_bc, in1=swo_sb)

                # ---- PV: outT[d, g] over all 256 chunks + current token ----
                v_dmas = []
                ps_pv = pspv_pool.tile([128, G], F32)
                for co in range(S // (128 * VCH)):
                    v_sb = v_pool.tile([128, VCH * D], F8E3, tag="vt")
                    vd = nc.sync.dma_start(out=v_sb, in_=v8[co])
                    v_dmas.append(vd)
                    for ci in range(VCH):
                        ch = co * VCH + ci
                        b, col = ch // 128, (ch % 128) * 4
                        nc.tensor.matmul(
                            ps_pv[:, :],
                            lhsT=v_sb[:, ci * D : (ci + 1) * D],
                            rhs=pprime_sb[b][:, col : col + 4],
                            start=(ch == 0),
                            stop=False,
                        )
                nc.tensor.matmul(
                    ps_pv[:, :], lhsT=vrow_sb, rhs=pcurf_sb, start=False, stop=True,
                )
                outn_sb = sm.tile([128, G], F16)
                outn_op = nc.vector.tensor_mul(out=outn_sb, in0=ps_pv, in1=bc_sb)

            # ---- o_proj: wo8 rides the sync FIFO raw right after v8 (no
            # deps, lands by ~V-end+6us); DVE upcasts it to fp16 during the
            # PV phase; ps_on gets all 8 free PSUM banks so the 32 MMs run
            # as one warm burst after outn.  Every chunk MM's waits merge
            # into a single DVE sem (upcasts and outn share the engine).
            with tc.tile_pool(name="pso", bufs=8, space=PS) as pso_pool:
                ofin_sb = sm.tile([1, HID], F32)
                for n in range(8):
                    wo8_sb = wo8_pool.tile([128, G * 512], I8, tag="wo8")
                    nc.sync.dma_start(out=wo8_sb, in_=wo8[n])
                    wo_sb = wo_pool.tile([128, G * 512], F16, tag="wo")
                    nc.vector.tensor_copy(out=wo_sb, in_=wo8_sb)
                    ps_on = pso_pool.tile([1, 512], F32, tag="on")
                    for g in range(G):
                        nc.tensor.matmul(
                            ps_on[:, :],
                            lhsT=outn_sb[:, g : g + 1],
                            rhs=wo_sb[:, g * 512 : (g + 1) * 512],
                            start=(g == 0),
                            stop=(g == G - 1),
                        )
                    nc.scalar.copy(out=ofin_sb[:, n * 512 : (n + 1) * 512], in_=ps_on)
            nc.gpsimd.dma_start(out=out[:, :], in_=ofin_sb)

    _reduce_dma_waits(nc)
    return nc


def _rope_fold(W, nheads, cos, sin, scale=1.0):
    """Fold RoPE rotation (and an optional scalar) into projection weights."""
    W = W.reshape(HID, nheads, D).astype(np.float32)
    half = D // 2
    Wr = np.empty_like(W)
    Wr[:, :, :half] = cos[:half] * W[:, :, :half] - sin[:half] * W[:, :, half:]
    Wr[:, :, half:] = cos[half:] * W[:, :, half:] + sin[half:] * W[:, :, :half]
    return (Wr * np.float32(scale)).reshape(HID, nheads * D)


def _prep_inputs(hidden_states, k_qx, k_scale, v_qx, v_scale, cos, sin, Wq, Wk, Wv, Wo):
    f16 = np.float16
    f8 = ml_dtypes.float8_e3m4
    hs = np.ascontiguousarray(hidden_states.reshape(HID)).astype(np.float32)
    cos = cos.astype(np.float32)
    sin = sin.astype(np.float32)
    Wq_f = _rope_fold(Wq, H, cos, sin)  # 1/sqrt(D) lives in ksc / scur scale
    Wk_f = _rope_fold(Wk, HKV, cos, sin)

    in_maps = []
    for c in range(NCORES):
        qcols = slice(G * c * D, G * (c + 1) * D)
        kvcols = slice(c * D, (c + 1) * D)
        Wsl = np.concatenate(
            [Wq_f[:, qcols], Wk_f[:, kvcols], Wv[:, kvcols].astype(np.float32)],
            axis=1,
        )  # [HID, 768]
        srow = np.maximum(np.abs(Wsl).max(axis=1) / 127.0, 1e-12)
        w8 = np.clip(np.rint(Wsl / srow[:, None]), -127, 127).astype(np.int8)
        wqkv = np.ascontiguousarray(
            w8.reshape(8, 4, 128, 768).transpose(0, 2, 1, 3)
        ).reshape(8, 128, 4 * 768)
        hsT = np.ascontiguousarray(
            (hs * srow * CHS).reshape(KC, 128).T
        ).astype(f16)

        kT = np.ascontiguousarray(k_qx[:, c, :].astype(np.int8).T)
        vcodes = v_qx[:, c, :].astype(np.float32)
        vf8 = (vcodes / F8S).astype(f8)
        v8a = np.ascontiguousarray(
            vf8.reshape(S // (128 * VCH), VCH, 128, D).transpose(0, 2, 1, 3)
        ).reshape(S // (128 * VCH), 128, VCH * D)
        # per-token LSQ refit of v_scale against the fp8-rounded codes
        vhat = vf8.astype(np.float32) * F8S
        adj = (vcodes * vhat).sum(1) / np.maximum((vhat * vhat).sum(1), 1e-9)
        ksc = np.ascontiguousarray(
            (k_scale[:, c, 0].astype(np.float32) * RSQD / CHS)
            .reshape(S // 128, 128).T
        ).astype(np.float32)
        vsc = np.ascontiguousarray(
            (v_scale[:, c, 0].astype(np.float32) * adj * F8S * 16384.0)
            .reshape(S // 128, 128).T
        ).astype(f16)

        Wol = Wo[G * c * D : G * (c + 1) * D, :].astype(np.float32)  # [512, HID]
        srow_o = np.maximum(np.abs(Wol).max(axis=1) / 127.0, 1e-12)
        wo8f = np.clip(np.rint(Wol / srow_o[:, None]), -127, 127).astype(np.int8)
        wo8 = np.ascontiguousarray(
            wo8f.reshape(G, 128, 8, 512).transpose(2, 1, 0, 3)
        ).reshape(8, 128, G * 512)
        swo2 = np.ascontiguousarray(
            (srow_o / 16384.0).reshape(G, 128).T
        ).astype(np.float32)

        in_maps.append(
            {"hsT": hsT, "swo2": swo2, "wqkv": wqkv, "kT": kT, "v8": v8a,
             "wo8": wo8, "ksc": ksc, "vsc": vsc}
        )
    return in_maps


def _run(in_maps, trace=False, **kw):
    if "nc" not in _CACHE:
        _CACHE["nc"] = _build_nc()
    return run_bass_kernel_spmd(
        _CACHE["nc"], in_maps, core_ids=list(range(NCORES)), trace=trace, **kw
    )


def kernel(hidden_states, k_qx, k_scale, v_qx, v_scale, cos, sin, Wq, Wk, Wv, Wo):
    in_maps = _prep_inputs(
        hidden_states, k_qx, k_scale, v_qx, v_scale, cos, sin, Wq, Wk, Wv, Wo
    )
    res = _run(in_maps)
    out = np.zeros((1, 1, HID), np.float32)
    for r in res.results:
        out += r["out"].reshape(1, 1, HID)
    return out
